# revision 15
# baseline (speedup 1.0000x reference)
"""GAU (Gated Attention Unit) Trainium2 kernel, 8-core SPMD.

Sharding: 2 cores per batch (B=4). Each core handles 1024 query rows of one
batch; the K/V path (LayerNorm + qk/v projections over the full 2048-row
sequence of that batch) is recomputed on both cores of a pair, which avoids
any cross-core collective. Host-side, each core's sequence is rotated so its
own query rows are always rows 0:1024 — attention is permutation-invariant
over the key/value index, so this is exact — which lets q/gate/out read
slices of the full-sequence tensors with one uniform SPMD program.

Compute dtype is bf16 on the TensorEngine (the GAU branch contributes
~1e-10 of the output magnitude relative to the residual, so bf16 is far
inside the error budget); LayerNorm statistics and the final residual add
are fp32. Weights are cast to bf16 once and staged through DRAM so the
transposed layouts are produced by a few large XBAR DMAs; the cast traffic
is interleaved into compute phases to fill DMA slack.
"""

from contextlib import ExitStack

import numpy as np

import concourse.bacc as bacc
import concourse.mybir as mybir
import concourse.tile as tile
from concourse.bass_utils import run_bass_kernel_spmd
from concourse.masks import make_identity

dt = mybir.dt
AF = mybir.ActivationFunctionType
ALU = mybir.AluOpType
AX = mybir.AxisListType

B, S, D = 4, 2048, 768
H = 1536          # v / gate each get H columns of the 2*H hidden projection
QK = 128
N_CORES = 8
SO = S // 2       # own query rows per core
EPS = 1e-5

_CACHE: dict = {}
SIM_COMPAT = False  # lower Silu as Sigmoid+mul (CoreSim has no Silu LUT)


def _build(flags, reps=1):
    use_bqk, use_bg, use_bv, use_bout, use_lnw, use_lnb = flags
    nc = bacc.Bacc("TRN2", target_bir_lowering=False, num_devices=N_CORES)

    XK = nc.declare_dram_parameter("xk", [S, D], dt.float32, isOutput=False)
    WH = nc.declare_dram_parameter("wh", [2 * H, D], dt.float32, isOutput=False)
    WQKD = nc.declare_dram_parameter("wqk", [QK, D], dt.float32, isOutput=False)
    WOUT = nc.declare_dram_parameter("wout", [D, H], dt.float32, isOutput=False)
    G0 = nc.declare_dram_parameter("g0", [QK, 1], dt.float32, isOutput=False)
    B0 = nc.declare_dram_parameter("b0", [QK, 1], dt.float32, isOutput=False)
    G1 = nc.declare_dram_parameter("g1", [QK, 1], dt.float32, isOutput=False)
    B1 = nc.declare_dram_parameter("b1", [QK, 1], dt.float32, isOutput=False)
    BQK = nc.declare_dram_parameter("bqk", [QK, 1], dt.float32, isOutput=False)
    BG = nc.declare_dram_parameter("bg", [128, 12], dt.float32, isOutput=False)
    BV = nc.declare_dram_parameter("bv", [1, H], dt.float32, isOutput=False)
    BOUT = nc.declare_dram_parameter("bout", [1, D], dt.float32, isOutput=False)
    LNW = nc.declare_dram_parameter("lnw", [1, D], dt.float32, isOutput=False)
    LNB = nc.declare_dram_parameter("lnb", [1, D], dt.float32, isOutput=False)
    OUT = nc.declare_dram_parameter("out", [SO, D], dt.float32, isOutput=True)

    ND = D // 128    # 6 d-tiles
    NH = H // 128    # 12 h-tiles
    NJ = S // 128    # 16 j-tiles
    NI = SO // 128   # 8 own-row tiles
    bf16, f32 = dt.bfloat16, dt.float32

    with tile.TileContext(nc) as tc:
      for _rep in range(reps):
        top = ExitStack()
        consts = top.enter_context(tc.tile_pool(name=f"consts{_rep}", bufs=1))
        ident = consts.tile([128, 128], bf16)
        make_identity(nc, ident[:])

        sc = {}
        for nm, hdl in (("g0", G0), ("b0", B0), ("g1", G1), ("b1", B1),
                        ("bqk", BQK)):
            t = consts.tile([128, 1], f32, tag=f"sc_{nm}", name=f"sc_{nm}")
            nc.sync.dma_start(t[:], hdl[:])
            sc[nm] = t
        bg_sb = consts.tile([128, 12], f32)
        nc.sync.dma_start(bg_sb[:], BG[:])

        ones_row = None

        def bcast_row(hdl, n, nm, dtype=bf16):
            nonlocal ones_row
            if ones_row is None:
                ones_row = consts.tile([1, 128], bf16, tag="ones_row",
                                       name="ones_row")
                nc.vector.memset(ones_row[:], 1.0)
            row_f = consts.tile([1, n], f32, tag=f"rf_{nm}", name=f"rf_{nm}")
            nc.sync.dma_start(row_f[:], hdl[:])
            row_b = consts.tile([1, n], bf16, tag=f"rb_{nm}", name=f"rb_{nm}")
            nc.vector.tensor_copy(row_b[:], row_f[:])
            out_t = consts.tile([128, n], dtype, tag=f"bc_{nm}", name=f"bc_{nm}")
            with tc.tile_pool(name=f"bcps_{nm}{_rep}", bufs=1, space="PSUM") as pp:
                for c0 in range(0, n, 512):
                    cw = min(512, n - c0)
                    ps = pp.tile([128, 512], f32, tag="ps", name=f"bcp_{nm}{c0}")
                    nc.tensor.matmul(ps[:, :cw], ones_row[:],
                                     row_b[:, c0:c0 + cw], start=True, stop=True)
                    nc.vector.tensor_copy(out_t[:, c0:c0 + cw], ps[:, :cw])
            return out_t

        bv_bc = bcast_row(BV, H, "bv") if use_bv else None
        bout_bc = bcast_row(BOUT, D, "bout", f32) if use_bout else None
        lnw_bc = bcast_row(LNW, D, "lnw") if use_lnw else None
        lnb_bc = bcast_row(LNB, D, "lnb") if use_lnb else None

        # bf16 weight copies staged through DRAM; the transposed layouts are
        # then produced by a few large XBAR DMAs.
        dram = top.enter_context(tc.tile_pool(name=f"dram{_rep}", bufs=1,
                                              space="DRAM"))
        WHB = dram.tile([2 * H, D], bf16, tag="whb", name="WHB")
        WOB = dram.tile([D, H], bf16, tag="wob", name="WOB")
        WQB = dram.tile([QK, D], bf16, tag="wqb", name="WQB")

        # long-lived pools, opened in LIFO-compatible close order
        es_vg = ExitStack()
        vg_pool = es_vg.enter_context(tc.tile_pool(name=f"VgT{_rep}", bufs=1))
        VgT = [vg_pool.tile([128, SO], bf16, tag=f"vg{h}", name=f"VgT{h}")
               for h in range(NH)]
        es_wo = ExitStack()
        wo_pool = es_wo.enter_context(tc.tile_pool(name=f"woT{_rep}", bufs=1))
        W_oT = [wo_pool.tile([128, D], bf16, tag=f"w{h}", name=f"WoT{h}")
                for h in range(NH)]
        es_nkv = ExitStack()
        nkv_pool = es_nkv.enter_context(tc.tile_pool(name=f"nkvT{_rep}", bufs=1))
        normT = [nkv_pool.tile([128, S], bf16, tag=f"n{d}", name=f"nkvT{d}")
                 for d in range(ND)]
        es_kq = ExitStack()
        kqp = es_kq.enter_context(tc.tile_pool(name=f"kq{_rep}", bufs=1))
        kT = kqp.tile([128, S], bf16, tag="kT")
        qT = kqp.tile([128, SO], bf16, tag="qT")
        es_at = ExitStack()
        at_pool = es_at.enter_context(tc.tile_pool(name=f"AT{_rep}", bufs=1))
        AT = [at_pool.tile([128, SO], bf16, tag=f"a{j}", name=f"AT{j}")
              for j in range(NJ)]
        es_v = ExitStack()
        v_pool = es_v.enter_context(tc.tile_pool(name=f"vnat{_rep}", bufs=1))
        v = [v_pool.tile([128, H], bf16, tag=f"v{j}", name=f"vnat{j}")
             for j in range(NJ)]

        # weight-cast staging (closed after the joint A^T/v loop)
        es_wc = ExitStack()
        wc = es_wc.enter_context(tc.tile_pool(name=f"wcast{_rep}", bufs=3))

        def cast_tile(srch, dsth, rt, c0, nm):
            wf = wc.tile([128, D], f32, tag="wf", name=f"wf{nm}{rt}_{c0}")
            nc.sync.dma_start(wf[:], srch[rt * 128:(rt + 1) * 128, c0:c0 + D])
            wb = wc.tile([128, D], bf16, tag="wb", name=f"wb{nm}{rt}_{c0}")
            nc.vector.tensor_copy(wb[:], wf[:])
            nc.sync.dma_start(dsth[rt * 128:(rt + 1) * 128, c0:c0 + D], wb[:])

        es_wqk = ExitStack()
        p_wqk = es_wqk.enter_context(tc.tile_pool(name=f"wqkT{_rep}", bufs=1))
        wqkT = [p_wqk.tile([128, 128], bf16, tag=f"q{d}", name=f"wqkT{d}")
                for d in range(ND)]
        cast_tile(WQKD, WQB, 0, 0, "q")
        for d in range(ND):
            nc.sync.dma_start(wqkT[d][:], WQB[:, d * 128:(d + 1) * 128],
                              transpose=True)
        # v-half casts drain during LayerNorm; gate-half and W_out casts
        # during the joint A^T / v-projection loop.
        cast_ln = [("h", rt, 0) for rt in range(12)]
        cast_at = ([("h", rt, 0) for rt in range(12, 24)] +
                   [("o", rt, c0) for rt in range(6) for c0 in (0, D)])

        def drain_cast(lst, k):
            for _ in range(k):
                if not lst:
                    return
                nm, rt, c0 = lst.pop(0)
                cast_tile(WH if nm == "h" else WOUT,
                          WHB if nm == "h" else WOB, rt, c0, nm)

        def silu(out_ap, in_ap, pool, nm, bias=None):
            if not SIM_COMPAT:
                if bias is None:
                    nc.scalar.activation(out_ap, in_ap, AF.Silu)
                else:
                    nc.scalar.activation(out_ap, in_ap, AF.Silu, bias=bias)
                return
            sig = pool.tile([128, 512], f32, tag="sig", name=f"sig_{nm}")
            if bias is None:
                nc.scalar.activation(sig[:], in_ap, AF.Sigmoid)
                nc.vector.scalar_tensor_tensor(out_ap, in_ap, 0.0, sig[:],
                                               op0=ALU.add, op1=ALU.mult)
            else:
                nc.scalar.activation(sig[:], in_ap, AF.Sigmoid, bias=bias)
                nc.vector.scalar_tensor_tensor(out_ap, in_ap, bias, sig[:],
                                               op0=ALU.add, op1=ALU.mult)

        # ---- Phase 1: LayerNorm + transpose + qk projection, per row group
        es_mm = ExitStack()
        mm_ps = es_mm.enter_context(tc.tile_pool(name=f"mm_ps{_rep}", bufs=4,
                                                 space="PSUM"))
        es_ln = ExitStack()
        xpool = es_ln.enter_context(tc.tile_pool(name=f"xin{_rep}", bufs=4))
        lnp = es_ln.enter_context(tc.tile_pool(name=f"lnwork{_rep}", bufs=2))
        nbp = es_ln.enter_context(tc.tile_pool(name=f"nbuf{_rep}", bufs=5))
        stat = es_ln.enter_context(tc.tile_pool(name=f"stat{_rep}", bufs=12))
        zb1 = es_ln.enter_context(tc.tile_pool(name=f"zbuf1{_rep}", bufs=3))
        tp_ps = es_ln.enter_context(
            tc.tile_pool(name=f"tp_ps{_rep}", bufs=4, space="PSUM"))
        for g in range(NJ // 4):
            nbs = []
            for k in range(4):
                nt = g * 4 + k
                xt = xpool.tile([128, D], f32, tag="x", name=f"x{nt}")
                nc.sync.dma_start(xt[:], XK[nt * 128:(nt + 1) * 128, :])
                if cast_ln:
                    drain_cast(cast_ln, 1)
                s = stat.tile([128, 1], f32, tag="s", name=f"s{nt}")
                nc.vector.reduce_sum(s[:], xt[:], axis=AX.X)
                sq = lnp.tile([128, D], f32, tag="sq", name=f"sq{nt}")
                ss = stat.tile([128, 1], f32, tag="ss", name=f"ss{nt}")
                nc.scalar.activation(sq[:], xt[:], AF.Square, accum_out=ss[:])
                mu = stat.tile([128, 1], f32, tag="mu", name=f"mu{nt}")
                nc.scalar.mul(mu[:], s[:], 1.0 / D)
                # var = E[x^2] + eps - mu^2
                vv = stat.tile([128, 1], f32, tag="vv", name=f"vv{nt}")
                nc.vector.tensor_scalar(vv[:], ss[:], 1.0 / D, EPS,
                                        ALU.mult, ALU.add)
                msq = stat.tile([128, 1], f32, tag="msq", name=f"msq{nt}")
                nc.vector.scalar_tensor_tensor(msq[:], mu[:], 1.0, mu[:],
                                               op0=ALU.mult, op1=ALU.mult)
                var = stat.tile([128, 1], f32, tag="var", name=f"var{nt}")
                nc.vector.tensor_sub(var[:], vv[:], msq[:])
                sr = stat.tile([128, 1], f32, tag="sr", name=f"sr{nt}")
                nc.scalar.sqrt(sr[:], var[:])
                rstd = stat.tile([128, 1], f32, tag="rstd", name=f"rstd{nt}")
                nc.vector.reciprocal(rstd[:], sr[:])
                nb = nbp.tile([128, D], bf16, tag="nb", name=f"nb{nt}")
                if use_lnw or use_lnb:
                    nrm = lnp.tile([128, D], f32, tag="nrm", name=f"nrm{nt}")
                    nc.vector.tensor_scalar(nrm[:], xt[:], mu[:], rstd[:],
                                            ALU.subtract, ALU.mult)
                    if use_lnw and use_lnb:
                        nc.vector.tensor_mul(nb[:], nrm[:], lnw_bc[:])
                        nc.vector.tensor_add(nb[:], nb[:], lnb_bc[:])
                    elif use_lnw:
                        nc.vector.tensor_mul(nb[:], nrm[:], lnw_bc[:])
                    else:
                        nc.vector.tensor_add(nb[:], nrm[:], lnb_bc[:])
                else:
                    nc.vector.tensor_scalar(nb[:], xt[:], mu[:], rstd[:],
                                            ALU.subtract, ALU.mult)
                nbs.append(nb)
            for d in range(ND):
                ps = tp_ps.tile([128, 512], bf16, tag="tp", name=f"tp{g}_{d}")
                for k in range(4):
                    nc.tensor.transpose(ps[:, k * 128:(k + 1) * 128],
                                        nbs[k][:, d * 128:(d + 1) * 128],
                                        ident[:])
                nc.scalar.copy(normT[d][:, g * 512:(g + 1) * 512], ps[:])
            # qk projection for this 512-row chunk
            c = g
            ps = mm_ps.tile([128, 512], f32, tag="ps", name=f"qkps{c}")
            for d in range(ND):
                nc.tensor.matmul(ps[:], wqkT[d][:],
                                 normT[d][:, c * 512:(c + 1) * 512],
                                 start=(d == 0), stop=(d == ND - 1))
            zs = zb1.tile([128, 512], bf16, tag="z", name=f"z{c}")
            silu(zs[:], ps[:], zb1, f"z{c}",
                 bias=sc["bqk"][:] if use_bqk else None)
            nc.vector.tensor_scalar(kT[:, c * 512:(c + 1) * 512], zs[:],
                                    sc["g1"][:], sc["b1"][:],
                                    ALU.mult, ALU.add)
            if c < SO // 512:
                nc.vector.tensor_scalar(qT[:, c * 512:(c + 1) * 512],
                                        zs[:], sc["g0"][:], sc["b0"][:],
                                        ALU.mult, ALU.add)
        drain_cast(cast_ln, len(cast_ln))
        es_ln.close()
        es_wqk.close()

        # W_vT now (v-half of WHB is complete) so the joint loop can start
        es_wv = ExitStack()
        p_wv = es_wv.enter_context(tc.tile_pool(name=f"wvT{_rep}", bufs=1))
        W_vT = [p_wv.tile([128, H], bf16, tag=f"v{d}", name=f"WvT{d}")
                for d in range(ND)]
        for d in range(ND):
            nc.sync.dma_start(W_vT[d][:], WHB[0:H, d * 128:(d + 1) * 128],
                              transpose=True)

        # ---- Phase 2: joint loop over j: A^T[j] and v[j]
        with tc.tile_pool(name=f"rbuf{_rep}", bufs=3) as rb, \
                tc.tile_pool(name=f"vraw{_rep}", bufs=2) as vrp:
            for j in range(NJ):
                drain_cast(cast_at, 2)
                for c in range(SO // 512):
                    ps = mm_ps.tile([128, 512], f32, tag="ps",
                                    name=f"aps{j}_{c}")
                    nc.tensor.matmul(ps[:], kT[:, j * 128:(j + 1) * 128],
                                     qT[:, c * 512:(c + 1) * 512],
                                     start=True, stop=True)
                    r = rb.tile([128, 512], bf16, tag="r", name=f"r{j}_{c}")
                    nc.scalar.activation(r[:], ps[:], AF.Relu, scale=1.0 / S)
                    nc.vector.tensor_mul(AT[j][:, c * 512:(c + 1) * 512],
                                         r[:], r[:])
                for c in range(H // 512):
                    ps = mm_ps.tile([128, 512], f32, tag="ps",
                                    name=f"vps{j}_{c}")
                    for d in range(ND):
                        nc.tensor.matmul(ps[:],
                                         normT[d][:, j * 128:(j + 1) * 128],
                                         W_vT[d][:, c * 512:(c + 1) * 512],
                                         start=(d == 0), stop=(d == ND - 1))
                    if use_bv:
                        raw = vrp.tile([128, 512], f32, tag="vr",
                                       name=f"vr{j}_{c}")
                        nc.vector.tensor_add(raw[:], ps[:],
                                             bv_bc[:, c * 512:(c + 1) * 512])
                        silu(v[j][:, c * 512:(c + 1) * 512], raw[:], vrp,
                             f"v{j}_{c}")
                    else:
                        silu(v[j][:, c * 512:(c + 1) * 512], ps[:], vrp,
                             f"v{j}_{c}")
        es_wv.close()
        drain_cast(cast_at, len(cast_at))
        for h in range(NH):
            nc.sync.dma_start(W_oT[h][:], WOB[:, h * 128:(h + 1) * 128],
                              transpose=True)
        es_wc.close()

        # ---- Phase 3: V^T[h,i] = sum_j v[j][:,h].T @ A^T[j][:,i]
        for h in range(NH):
            for c in range(SO // 512):
                ps = mm_ps.tile([128, 512], f32, tag="ps", name=f"Vps{h}_{c}")
                for j in range(NJ):
                    nc.tensor.matmul(ps[:], v[j][:, h * 128:(h + 1) * 128],
                                     AT[j][:, c * 512:(c + 1) * 512],
                                     start=(j == 0), stop=(j == NJ - 1))
                nc.vector.tensor_copy(VgT[h][:, c * 512:(c + 1) * 512], ps[:])

        # ---- Phase 4: gate^T chunkwise, multiply into VgT
        es_wg = ExitStack()
        p_wg = es_wg.enter_context(tc.tile_pool(name=f"wgT{_rep}", bufs=1))
        W_gT = [p_wg.tile([128, H], bf16, tag=f"g{d}", name=f"WgT{d}")
                for d in range(ND)]
        for d in range(ND):
            nc.sync.dma_start(W_gT[d][:], WHB[H:2 * H, d * 128:(d + 1) * 128],
                              transpose=True)
        with tc.tile_pool(name=f"zg{_rep}", bufs=3) as zgp:
            for h in range(NH):
                for c in range(SO // 512):
                    ps = mm_ps.tile([128, 512], f32, tag="ps",
                                    name=f"gps{h}_{c}")
                    for d in range(ND):
                        nc.tensor.matmul(ps[:],
                                         W_gT[d][:, h * 128:(h + 1) * 128],
                                         normT[d][:, c * 512:(c + 1) * 512],
                                         start=(d == 0), stop=(d == ND - 1))
                    zg = zgp.tile([128, 512], bf16, tag="zg",
                                  name=f"zg{h}_{c}")
                    silu(zg[:], ps[:], zgp, f"zg{h}_{c}",
                         bias=bg_sb[:, h:h + 1] if use_bg else None)
                    nc.vector.tensor_mul(VgT[h][:, c * 512:(c + 1) * 512],
                                         VgT[h][:, c * 512:(c + 1) * 512],
                                         zg[:])
        es_wg.close()
        es_v.close()
        es_at.close()
        es_kq.close()
        es_nkv.close()

        # ---- Phase 5: out = VgT.T-blocks @ W_oT + x (+ b_out)
        with tc.tile_pool(name=f"xq2{_rep}", bufs=3) as xp2, \
                tc.tile_pool(name=f"obuf{_rep}", bufs=3) as op:
            for it in range(NI):
                xqt = xp2.tile([128, D], f32, tag="xq", name=f"xq{it}")
                nc.sync.dma_start(xqt[:], XK[it * 128:(it + 1) * 128, :])
                ob = op.tile([128, D], f32, tag="ob", name=f"ob{it}")
                cw = D // 2  # 384
                for c in range(2):
                    ps = mm_ps.tile([128, 512], f32, tag="ps",
                                    name=f"ops{it}_{c}")
                    for h in range(NH):
                        nc.tensor.matmul(ps[:, :cw],
                                         VgT[h][:, it * 128:(it + 1) * 128],
                                         W_oT[h][:, c * cw:(c + 1) * cw],
                                         start=(h == 0), stop=(h == NH - 1))
                    nc.vector.tensor_add(ob[:, c * cw:(c + 1) * cw],
                                         ps[:, :cw], xqt[:, c * cw:(c + 1) * cw])
                    if use_bout:
                        nc.vector.tensor_add(ob[:, c * cw:(c + 1) * cw],
                                             ob[:, c * cw:(c + 1) * cw],
                                             bout_bc[:, c * cw:(c + 1) * cw])
                nc.sync.dma_start(OUT[it * 128:(it + 1) * 128, :], ob[:])
        es_mm.close()
        es_wo.close()
        es_vg.close()
        top.close()

    nc.finalize()
    return nc


def _prep_in_maps(x, ln_w, ln_b, W_hidden, b_hidden, W_qk, b_qk, gamma, beta,
                  W_out, b_out):
    f32 = np.float32
    c = np.ascontiguousarray
    shared = {
        "wh": c(W_hidden, dtype=f32),
        "wqk": c(W_qk, dtype=f32),
        "wout": c(W_out, dtype=f32),
        "g0": c(gamma[0].reshape(QK, 1), dtype=f32),
        "b0": c(beta[0].reshape(QK, 1), dtype=f32),
        "g1": c(gamma[1].reshape(QK, 1), dtype=f32),
        "b1": c(beta[1].reshape(QK, 1), dtype=f32),
        "bqk": c(b_qk.reshape(QK, 1), dtype=f32),
        "bg": c(b_hidden[H:].reshape(12, 128).T, dtype=f32),
        "bv": c(b_hidden[:H].reshape(1, H), dtype=f32),
        "bout": c(b_out.reshape(1, D), dtype=f32),
        "lnw": c(ln_w.reshape(1, D), dtype=f32),
        "lnb": c(ln_b.reshape(1, D), dtype=f32),
    }
    in_maps = []
    for core in range(N_CORES):
        b, hf = core // 2, core % 2
        m = dict(shared)
        if hf == 0:
            m["xk"] = c(x[b], dtype=f32)
        else:
            m["xk"] = c(np.concatenate([x[b, SO:], x[b, :SO]], axis=0),
                        dtype=f32)
        in_maps.append(m)
    return in_maps


def _flags(ln_w, ln_b, b_hidden, b_qk, b_out):
    return (
        bool(np.any(b_qk)),
        bool(np.any(b_hidden[H:])),
        bool(np.any(b_hidden[:H])),
        bool(np.any(b_out)),
        bool(np.any(ln_w != 1.0)),
        bool(np.any(ln_b)),
    )


def get_program(inputs):
    flags = _flags(inputs["ln_w"], inputs["ln_b"], inputs["b_hidden"],
                   inputs["b_qk"], inputs["b_out"])
    key = (flags, SIM_COMPAT)
    if key not in _CACHE:
        _CACHE[key] = _build(flags)
    return _CACHE[key]


def kernel(x, ln_w, ln_b, W_hidden, b_hidden, W_qk, b_qk, gamma, beta,
           W_out, b_out):
    inputs = dict(x=np.asarray(x), ln_w=np.asarray(ln_w),
                  ln_b=np.asarray(ln_b), W_hidden=np.asarray(W_hidden),
                  b_hidden=np.asarray(b_hidden), W_qk=np.asarray(W_qk),
                  b_qk=np.asarray(b_qk), gamma=np.asarray(gamma),
                  beta=np.asarray(beta), W_out=np.asarray(W_out),
                  b_out=np.asarray(b_out))
    nc = get_program(inputs)
    in_maps = _prep_in_maps(**inputs)
    res = run_bass_kernel_spmd(nc, in_maps, core_ids=list(range(N_CORES)),
                               trace=False)
    out = np.empty((B, S, D), np.float32)
    for core in range(N_CORES):
        b, hf = core // 2, core % 2
        out[b, hf * SO:(hf + 1) * SO] = res.results[core]["out"]
    return out


# revision 16
# speedup vs baseline: 30.8611x; 30.8611x over previous
"""GAU (Gated Attention Unit) Trainium2 kernel, 8-core SPMD.

Sharding: 2 cores per batch (B=4). Each core handles 1024 query rows of one
batch; the K/V path (LayerNorm + qk/v projections over the full 2048-row
sequence of that batch) is recomputed on both cores of a pair, which avoids
any cross-core collective. Host-side, each core's sequence is rotated so its
own query rows are always rows 0:1024 — attention is permutation-invariant
over the key/value index, so this is exact — which lets q/gate/out read
slices of the full-sequence tensors with one uniform SPMD program.

Compute dtype is bf16 on the TensorEngine (the GAU branch contributes
~1e-10 of the output magnitude relative to the residual, so bf16 is far
inside the error budget); LayerNorm statistics and the final residual add
are fp32. Weights are cast to bf16 once and staged through DRAM so the
transposed layouts are produced by a few large XBAR DMAs; the cast traffic
is interleaved into compute phases to fill DMA slack.
"""

from contextlib import ExitStack

import numpy as np

import concourse.bacc as bacc
import concourse.mybir as mybir
import concourse.tile as tile
from concourse.bass_utils import run_bass_kernel_spmd
from concourse.masks import make_identity

dt = mybir.dt
AF = mybir.ActivationFunctionType
ALU = mybir.AluOpType
AX = mybir.AxisListType

B, S, D = 4, 2048, 768
H = 1536          # v / gate each get H columns of the 2*H hidden projection
QK = 128
N_CORES = 8
SO = S // 2       # own query rows per core
EPS = 1e-5

_CACHE: dict = {}
SIM_COMPAT = False  # lower Silu as Sigmoid+mul (CoreSim has no Silu LUT)


def _build(flags, reps=1):
    use_bqk, use_bg, use_bv, use_bout, use_lnw, use_lnb = flags
    nc = bacc.Bacc("TRN2", target_bir_lowering=False, num_devices=N_CORES)

    XK = nc.declare_dram_parameter("xk", [S, D], dt.float32, isOutput=False)
    WH = nc.declare_dram_parameter("wh", [2 * H, D], dt.float32, isOutput=False)
    WQKD = nc.declare_dram_parameter("wqk", [QK, D], dt.float32, isOutput=False)
    WOUT = nc.declare_dram_parameter("wout", [D, H], dt.float32, isOutput=False)
    G0 = nc.declare_dram_parameter("g0", [QK, 1], dt.float32, isOutput=False)
    B0 = nc.declare_dram_parameter("b0", [QK, 1], dt.float32, isOutput=False)
    G1 = nc.declare_dram_parameter("g1", [QK, 1], dt.float32, isOutput=False)
    B1 = nc.declare_dram_parameter("b1", [QK, 1], dt.float32, isOutput=False)
    BQK = nc.declare_dram_parameter("bqk", [QK, 1], dt.float32, isOutput=False)
    BG = nc.declare_dram_parameter("bg", [128, 12], dt.float32, isOutput=False)
    BV = nc.declare_dram_parameter("bv", [1, H], dt.float32, isOutput=False)
    BOUT = nc.declare_dram_parameter("bout", [1, D], dt.float32, isOutput=False)
    LNW = nc.declare_dram_parameter("lnw", [1, D], dt.float32, isOutput=False)
    LNB = nc.declare_dram_parameter("lnb", [1, D], dt.float32, isOutput=False)
    OUT = nc.declare_dram_parameter("out", [SO, D], dt.float32, isOutput=True)

    ND = D // 128    # 6 d-tiles
    NH = H // 128    # 12 h-tiles
    NJ = S // 128    # 16 j-tiles
    NI = SO // 128   # 8 own-row tiles
    bf16, f32 = dt.bfloat16, dt.float32

    with tile.TileContext(nc) as tc:
      for _rep in range(reps):
        top = ExitStack()
        consts = top.enter_context(tc.tile_pool(name=f"consts{_rep}", bufs=1))
        ident = consts.tile([128, 128], bf16)
        make_identity(nc, ident[:])

        sc = {}
        for nm, hdl in (("g0", G0), ("b0", B0), ("g1", G1), ("b1", B1),
                        ("bqk", BQK)):
            t = consts.tile([128, 1], f32, tag=f"sc_{nm}", name=f"sc_{nm}")
            nc.sync.dma_start(t[:], hdl[:])
            sc[nm] = t
        bg_sb = consts.tile([128, 12], f32)
        nc.sync.dma_start(bg_sb[:], BG[:])

        ones_row = None

        def bcast_row(hdl, n, nm, dtype=bf16):
            nonlocal ones_row
            if ones_row is None:
                ones_row = consts.tile([1, 128], bf16, tag="ones_row",
                                       name="ones_row")
                nc.vector.memset(ones_row[:], 1.0)
            row_f = consts.tile([1, n], f32, tag=f"rf_{nm}", name=f"rf_{nm}")
            nc.sync.dma_start(row_f[:], hdl[:])
            row_b = consts.tile([1, n], bf16, tag=f"rb_{nm}", name=f"rb_{nm}")
            nc.vector.tensor_copy(row_b[:], row_f[:])
            out_t = consts.tile([128, n], dtype, tag=f"bc_{nm}", name=f"bc_{nm}")
            with tc.tile_pool(name=f"bcps_{nm}{_rep}", bufs=1, space="PSUM") as pp:
                for c0 in range(0, n, 512):
                    cw = min(512, n - c0)
                    ps = pp.tile([128, 512], f32, tag="ps", name=f"bcp_{nm}{c0}")
                    nc.tensor.matmul(ps[:, :cw], ones_row[:],
                                     row_b[:, c0:c0 + cw], start=True, stop=True)
                    nc.vector.tensor_copy(out_t[:, c0:c0 + cw], ps[:, :cw])
            return out_t

        bv_bc = bcast_row(BV, H, "bv") if use_bv else None
        bout_bc = bcast_row(BOUT, D, "bout", f32) if use_bout else None
        lnw_bc = bcast_row(LNW, D, "lnw") if use_lnw else None
        lnb_bc = bcast_row(LNB, D, "lnb") if use_lnb else None

        # bf16 weight copies staged through DRAM; the transposed layouts are
        # then produced by a few large XBAR DMAs.
        dram = top.enter_context(tc.tile_pool(name=f"dram{_rep}", bufs=1,
                                              space="DRAM"))
        WHB = dram.tile([2 * H, D], bf16, tag="whb", name="WHB")
        WOB = dram.tile([D, H], bf16, tag="wob", name="WOB")
        WQB = dram.tile([QK, D], bf16, tag="wqb", name="WQB")

        # long-lived pools, opened in LIFO-compatible close order
        es_vg = ExitStack()
        vg_pool = es_vg.enter_context(tc.tile_pool(name=f"VgT{_rep}", bufs=1))
        VgT = [vg_pool.tile([128, SO], bf16, tag=f"vg{h}", name=f"VgT{h}")
               for h in range(NH)]
        es_wo = ExitStack()
        wo_pool = es_wo.enter_context(tc.tile_pool(name=f"woT{_rep}", bufs=1))
        W_oT = [wo_pool.tile([128, D], bf16, tag=f"w{h}", name=f"WoT{h}")
                for h in range(NH)]
        es_nkv = ExitStack()
        nkv_pool = es_nkv.enter_context(tc.tile_pool(name=f"nkvT{_rep}", bufs=1))
        normT = [nkv_pool.tile([128, S], bf16, tag=f"n{d}", name=f"nkvT{d}")
                 for d in range(ND)]
        es_kq = ExitStack()
        kqp = es_kq.enter_context(tc.tile_pool(name=f"kq{_rep}", bufs=1))
        kT = kqp.tile([128, S], bf16, tag="kT")
        qT = kqp.tile([128, SO], bf16, tag="qT")
        es_at = ExitStack()
        at_pool = es_at.enter_context(tc.tile_pool(name=f"AT{_rep}", bufs=1))
        AT = [at_pool.tile([128, SO], bf16, tag=f"a{j}", name=f"AT{j}")
              for j in range(NJ)]
        es_v = ExitStack()
        v_pool = es_v.enter_context(tc.tile_pool(name=f"vnat{_rep}", bufs=1))
        v = [v_pool.tile([128, H], bf16, tag=f"v{j}", name=f"vnat{j}")
             for j in range(NJ)]

        # weight-cast staging (closed after the joint A^T/v loop)
        es_wc = ExitStack()
        wc = es_wc.enter_context(tc.tile_pool(name=f"wcast{_rep}", bufs=3))

        def cast_tile(srch, dsth, rt, c0, nm):
            wf = wc.tile([128, D], f32, tag="wf", name=f"wf{nm}{rt}_{c0}")
            nc.sync.dma_start(wf[:], srch[rt * 128:(rt + 1) * 128, c0:c0 + D])
            wb = wc.tile([128, D], bf16, tag="wb", name=f"wb{nm}{rt}_{c0}")
            nc.vector.tensor_copy(wb[:], wf[:])
            nc.sync.dma_start(dsth[rt * 128:(rt + 1) * 128, c0:c0 + D], wb[:])

        es_wqk = ExitStack()
        p_wqk = es_wqk.enter_context(tc.tile_pool(name=f"wqkT{_rep}", bufs=1))
        wqkT = [p_wqk.tile([128, 128], bf16, tag=f"q{d}", name=f"wqkT{d}")
                for d in range(ND)]
        cast_tile(WQKD, WQB, 0, 0, "q")
        for d in range(ND):
            nc.sync.dma_start(wqkT[d][:], WQB[:, d * 128:(d + 1) * 128],
                              transpose=True)
        # v-half casts drain during LayerNorm; gate-half and W_out casts
        # during the joint A^T / v-projection loop.
        cast_ln = [("h", rt, 0) for rt in range(12)]
        cast_at = ([("h", rt, 0) for rt in range(12, 24)] +
                   [("o", rt, c0) for rt in range(6) for c0 in (0, D)])

        def drain_cast(lst, k):
            for _ in range(k):
                if not lst:
                    return
                nm, rt, c0 = lst.pop(0)
                cast_tile(WH if nm == "h" else WOUT,
                          WHB if nm == "h" else WOB, rt, c0, nm)

        def silu(out_ap, in_ap, pool, nm, bias=None):
            if not SIM_COMPAT:
                if bias is None:
                    nc.scalar.activation(out_ap, in_ap, AF.Silu)
                else:
                    nc.scalar.activation(out_ap, in_ap, AF.Silu, bias=bias)
                return
            sig = pool.tile([128, 512], f32, tag="sig", name=f"sig_{nm}")
            if bias is None:
                nc.scalar.activation(sig[:], in_ap, AF.Sigmoid)
                nc.vector.scalar_tensor_tensor(out_ap, in_ap, 0.0, sig[:],
                                               op0=ALU.add, op1=ALU.mult)
            else:
                nc.scalar.activation(sig[:], in_ap, AF.Sigmoid, bias=bias)
                nc.vector.scalar_tensor_tensor(out_ap, in_ap, bias, sig[:],
                                               op0=ALU.add, op1=ALU.mult)

        # ---- Phase 1: LayerNorm + transpose + qk projection, per row group
        es_mm = ExitStack()
        mm_ps = es_mm.enter_context(tc.tile_pool(name=f"mm_ps{_rep}", bufs=4,
                                                 space="PSUM"))
        es_ln = ExitStack()
        xpool = es_ln.enter_context(tc.tile_pool(name=f"xin{_rep}", bufs=4))
        lnp = es_ln.enter_context(tc.tile_pool(name=f"lnwork{_rep}", bufs=2))
        nbp = es_ln.enter_context(tc.tile_pool(name=f"nbuf{_rep}", bufs=5))
        stat = es_ln.enter_context(tc.tile_pool(name=f"stat{_rep}", bufs=12))
        zb1 = es_ln.enter_context(tc.tile_pool(name=f"zbuf1{_rep}", bufs=3))
        tp_ps = es_ln.enter_context(
            tc.tile_pool(name=f"tp_ps{_rep}", bufs=4, space="PSUM"))
        for g in range(NJ // 4):
            nbs = []
            for k in range(4):
                nt = g * 4 + k
                xt = xpool.tile([128, D], f32, tag="x", name=f"x{nt}")
                nc.sync.dma_start(xt[:], XK[nt * 128:(nt + 1) * 128, :])
                if cast_ln:
                    drain_cast(cast_ln, 2)
                s = stat.tile([128, 1], f32, tag="s", name=f"s{nt}")
                nc.vector.reduce_sum(s[:], xt[:], axis=AX.X)
                sq = lnp.tile([128, D], f32, tag="sq", name=f"sq{nt}")
                ss = stat.tile([128, 1], f32, tag="ss", name=f"ss{nt}")
                nc.scalar.activation(sq[:], xt[:], AF.Square, accum_out=ss[:])
                mu = stat.tile([128, 1], f32, tag="mu", name=f"mu{nt}")
                nc.scalar.mul(mu[:], s[:], 1.0 / D)
                # var = E[x^2] + eps - mu^2
                vv = stat.tile([128, 1], f32, tag="vv", name=f"vv{nt}")
                nc.vector.tensor_scalar(vv[:], ss[:], 1.0 / D, EPS,
                                        ALU.mult, ALU.add)
                msq = stat.tile([128, 1], f32, tag="msq", name=f"msq{nt}")
                nc.vector.scalar_tensor_tensor(msq[:], mu[:], 1.0, mu[:],
                                               op0=ALU.mult, op1=ALU.mult)
                var = stat.tile([128, 1], f32, tag="var", name=f"var{nt}")
                nc.vector.tensor_sub(var[:], vv[:], msq[:])
                sr = stat.tile([128, 1], f32, tag="sr", name=f"sr{nt}")
                nc.scalar.sqrt(sr[:], var[:])
                rstd = stat.tile([128, 1], f32, tag="rstd", name=f"rstd{nt}")
                nc.vector.reciprocal(rstd[:], sr[:])
                nb = nbp.tile([128, D], bf16, tag="nb", name=f"nb{nt}")
                if use_lnw or use_lnb:
                    nrm = lnp.tile([128, D], f32, tag="nrm", name=f"nrm{nt}")
                    nc.vector.tensor_scalar(nrm[:], xt[:], mu[:], rstd[:],
                                            ALU.subtract, ALU.mult)
                    if use_lnw and use_lnb:
                        nc.vector.tensor_mul(nb[:], nrm[:], lnw_bc[:])
                        nc.vector.tensor_add(nb[:], nb[:], lnb_bc[:])
                    elif use_lnw:
                        nc.vector.tensor_mul(nb[:], nrm[:], lnw_bc[:])
                    else:
                        nc.vector.tensor_add(nb[:], nrm[:], lnb_bc[:])
                else:
                    nc.vector.tensor_scalar(nb[:], xt[:], mu[:], rstd[:],
                                            ALU.subtract, ALU.mult)
                nbs.append(nb)
            for d in range(ND):
                ps = tp_ps.tile([128, 512], bf16, tag="tp", name=f"tp{g}_{d}")
                for k in range(4):
                    nc.tensor.transpose(ps[:, k * 128:(k + 1) * 128],
                                        nbs[k][:, d * 128:(d + 1) * 128],
                                        ident[:])
                if d % 2 == 0:
                    nc.scalar.copy(normT[d][:, g * 512:(g + 1) * 512], ps[:])
                else:
                    nc.vector.tensor_copy(normT[d][:, g * 512:(g + 1) * 512],
                                          ps[:])
            # qk projection for this 512-row chunk
            c = g
            ps = mm_ps.tile([128, 512], f32, tag="ps", name=f"qkps{c}")
            for d in range(ND):
                nc.tensor.matmul(ps[:], wqkT[d][:],
                                 normT[d][:, c * 512:(c + 1) * 512],
                                 start=(d == 0), stop=(d == ND - 1))
            zs = zb1.tile([128, 512], bf16, tag="z", name=f"z{c}")
            silu(zs[:], ps[:], zb1, f"z{c}",
                 bias=sc["bqk"][:] if use_bqk else None)
            nc.vector.tensor_scalar(kT[:, c * 512:(c + 1) * 512], zs[:],
                                    sc["g1"][:], sc["b1"][:],
                                    ALU.mult, ALU.add)
            if c < SO // 512:
                nc.vector.tensor_scalar(qT[:, c * 512:(c + 1) * 512],
                                        zs[:], sc["g0"][:], sc["b0"][:],
                                        ALU.mult, ALU.add)
        drain_cast(cast_ln, len(cast_ln))
        es_ln.close()
        es_wqk.close()

        # W_vT now (v-half of WHB is complete) so the joint loop can start
        es_wv = ExitStack()
        p_wv = es_wv.enter_context(tc.tile_pool(name=f"wvT{_rep}", bufs=1))
        W_vT = [p_wv.tile([128, H], bf16, tag=f"v{d}", name=f"WvT{d}")
                for d in range(ND)]
        for d in range(ND):
            nc.sync.dma_start(W_vT[d][:], WHB[0:H, d * 128:(d + 1) * 128],
                              transpose=True)

        # ---- Phase 2: joint loop over j: A^T[j] and v[j]
        with tc.tile_pool(name=f"rbuf{_rep}", bufs=3) as rb, \
                tc.tile_pool(name=f"vraw{_rep}", bufs=2) as vrp:
            for j in range(NJ):
                drain_cast(cast_at, 2)
                for c in range(SO // 512):
                    ps = mm_ps.tile([128, 512], f32, tag="ps",
                                    name=f"aps{j}_{c}")
                    nc.tensor.matmul(ps[:], kT[:, j * 128:(j + 1) * 128],
                                     qT[:, c * 512:(c + 1) * 512],
                                     start=True, stop=True)
                    r = rb.tile([128, 512], bf16, tag="r", name=f"r{j}_{c}")
                    nc.scalar.activation(r[:], ps[:], AF.Relu, scale=1.0 / S)
                    nc.vector.tensor_mul(AT[j][:, c * 512:(c + 1) * 512],
                                         r[:], r[:])
                for c in range(H // 512):
                    ps = mm_ps.tile([128, 512], f32, tag="ps",
                                    name=f"vps{j}_{c}")
                    for d in range(ND):
                        nc.tensor.matmul(ps[:],
                                         normT[d][:, j * 128:(j + 1) * 128],
                                         W_vT[d][:, c * 512:(c + 1) * 512],
                                         start=(d == 0), stop=(d == ND - 1))
                    if use_bv:
                        raw = vrp.tile([128, 512], f32, tag="vr",
                                       name=f"vr{j}_{c}")
                        nc.vector.tensor_add(raw[:], ps[:],
                                             bv_bc[:, c * 512:(c + 1) * 512])
                        silu(v[j][:, c * 512:(c + 1) * 512], raw[:], vrp,
                             f"v{j}_{c}")
                    else:
                        silu(v[j][:, c * 512:(c + 1) * 512], ps[:], vrp,
                             f"v{j}_{c}")
        es_wv.close()
        drain_cast(cast_at, len(cast_at))
        for h in range(NH):
            nc.sync.dma_start(W_oT[h][:], WOB[:, h * 128:(h + 1) * 128],
                              transpose=True)
        es_wc.close()

        # ---- Phase 3: V^T[h,i] = sum_j v[j][:,h].T @ A^T[j][:,i]
        for h in range(NH):
            for c in range(SO // 512):
                ps = mm_ps.tile([128, 512], f32, tag="ps", name=f"Vps{h}_{c}")
                for j in range(NJ):
                    nc.tensor.matmul(ps[:], v[j][:, h * 128:(h + 1) * 128],
                                     AT[j][:, c * 512:(c + 1) * 512],
                                     start=(j == 0), stop=(j == NJ - 1))
                nc.vector.tensor_copy(VgT[h][:, c * 512:(c + 1) * 512], ps[:])

        # ---- Phase 4: gate^T chunkwise, multiply into VgT
        es_wg = ExitStack()
        p_wg = es_wg.enter_context(tc.tile_pool(name=f"wgT{_rep}", bufs=1))
        W_gT = [p_wg.tile([128, H], bf16, tag=f"g{d}", name=f"WgT{d}")
                for d in range(ND)]
        for d in range(ND):
            nc.sync.dma_start(W_gT[d][:], WHB[H:2 * H, d * 128:(d + 1) * 128],
                              transpose=True)
        with tc.tile_pool(name=f"zg{_rep}", bufs=3) as zgp:
            for h in range(NH):
                for c in range(SO // 512):
                    ps = mm_ps.tile([128, 512], f32, tag="ps",
                                    name=f"gps{h}_{c}")
                    for d in range(ND):
                        nc.tensor.matmul(ps[:],
                                         W_gT[d][:, h * 128:(h + 1) * 128],
                                         normT[d][:, c * 512:(c + 1) * 512],
                                         start=(d == 0), stop=(d == ND - 1))
                    zg = zgp.tile([128, 512], bf16, tag="zg",
                                  name=f"zg{h}_{c}")
                    silu(zg[:], ps[:], zgp, f"zg{h}_{c}",
                         bias=bg_sb[:, h:h + 1] if use_bg else None)
                    nc.vector.tensor_mul(VgT[h][:, c * 512:(c + 1) * 512],
                                         VgT[h][:, c * 512:(c + 1) * 512],
                                         zg[:])
        es_wg.close()
        es_v.close()
        es_at.close()
        es_kq.close()
        es_nkv.close()

        # ---- Phase 5: out = VgT.T-blocks @ W_oT + x (+ b_out)
        with tc.tile_pool(name=f"xq2{_rep}", bufs=3) as xp2, \
                tc.tile_pool(name=f"obuf{_rep}", bufs=3) as op:
            for it in range(NI):
                xqt = xp2.tile([128, D], f32, tag="xq", name=f"xq{it}")
                nc.sync.dma_start(xqt[:], XK[it * 128:(it + 1) * 128, :])
                ob = op.tile([128, D], f32, tag="ob", name=f"ob{it}")
                cw = D // 2  # 384
                for c in range(2):
                    ps = mm_ps.tile([128, 512], f32, tag="ps",
                                    name=f"ops{it}_{c}")
                    for h in range(NH):
                        nc.tensor.matmul(ps[:, :cw],
                                         VgT[h][:, it * 128:(it + 1) * 128],
                                         W_oT[h][:, c * cw:(c + 1) * cw],
                                         start=(h == 0), stop=(h == NH - 1))
                    nc.vector.tensor_add(ob[:, c * cw:(c + 1) * cw],
                                         ps[:, :cw], xqt[:, c * cw:(c + 1) * cw])
                    if use_bout:
                        nc.vector.tensor_add(ob[:, c * cw:(c + 1) * cw],
                                             ob[:, c * cw:(c + 1) * cw],
                                             bout_bc[:, c * cw:(c + 1) * cw])
                nc.sync.dma_start(OUT[it * 128:(it + 1) * 128, :], ob[:])
        es_mm.close()
        es_wo.close()
        es_vg.close()
        top.close()

    nc.finalize()
    return nc


def _prep_in_maps(x, ln_w, ln_b, W_hidden, b_hidden, W_qk, b_qk, gamma, beta,
                  W_out, b_out):
    f32 = np.float32
    c = np.ascontiguousarray
    shared = {
        "wh": c(W_hidden, dtype=f32),
        "wqk": c(W_qk, dtype=f32),
        "wout": c(W_out, dtype=f32),
        "g0": c(gamma[0].reshape(QK, 1), dtype=f32),
        "b0": c(beta[0].reshape(QK, 1), dtype=f32),
        "g1": c(gamma[1].reshape(QK, 1), dtype=f32),
        "b1": c(beta[1].reshape(QK, 1), dtype=f32),
        "bqk": c(b_qk.reshape(QK, 1), dtype=f32),
        "bg": c(b_hidden[H:].reshape(12, 128).T, dtype=f32),
        "bv": c(b_hidden[:H].reshape(1, H), dtype=f32),
        "bout": c(b_out.reshape(1, D), dtype=f32),
        "lnw": c(ln_w.reshape(1, D), dtype=f32),
        "lnb": c(ln_b.reshape(1, D), dtype=f32),
    }
    in_maps = []
    for core in range(N_CORES):
        b, hf = core // 2, core % 2
        m = dict(shared)
        if hf == 0:
            m["xk"] = c(x[b], dtype=f32)
        else:
            m["xk"] = c(np.concatenate([x[b, SO:], x[b, :SO]], axis=0),
                        dtype=f32)
        in_maps.append(m)
    return in_maps


def _flags(ln_w, ln_b, b_hidden, b_qk, b_out):
    return (
        bool(np.any(b_qk)),
        bool(np.any(b_hidden[H:])),
        bool(np.any(b_hidden[:H])),
        bool(np.any(b_out)),
        bool(np.any(ln_w != 1.0)),
        bool(np.any(ln_b)),
    )


def get_program(inputs):
    flags = _flags(inputs["ln_w"], inputs["ln_b"], inputs["b_hidden"],
                   inputs["b_qk"], inputs["b_out"])
    key = (flags, SIM_COMPAT)
    if key not in _CACHE:
        _CACHE[key] = _build(flags)
    return _CACHE[key]


def kernel(x, ln_w, ln_b, W_hidden, b_hidden, W_qk, b_qk, gamma, beta,
           W_out, b_out):
    inputs = dict(x=np.asarray(x), ln_w=np.asarray(ln_w),
                  ln_b=np.asarray(ln_b), W_hidden=np.asarray(W_hidden),
                  b_hidden=np.asarray(b_hidden), W_qk=np.asarray(W_qk),
                  b_qk=np.asarray(b_qk), gamma=np.asarray(gamma),
                  beta=np.asarray(beta), W_out=np.asarray(W_out),
                  b_out=np.asarray(b_out))
    nc = get_program(inputs)
    in_maps = _prep_in_maps(**inputs)
    res = run_bass_kernel_spmd(nc, in_maps, core_ids=list(range(N_CORES)),
                               trace=False)
    out = np.empty((B, S, D), np.float32)
    for core in range(N_CORES):
        b, hf = core // 2, core % 2
        out[b, hf * SO:(hf + 1) * SO] = res.results[core]["out"]
    return out


# revision 20
# speedup vs baseline: 16580.8923x; 537.2755x over previous
"""GAU (Gated Attention Unit) Trainium2 kernel, 8-core SPMD.

Sharding: 2 cores per batch (B=4). Each core handles 1024 query rows of one
batch; the K/V path (LayerNorm + qk/v projections over the full 2048-row
sequence of that batch) is recomputed on both cores of a pair, which avoids
any cross-core collective. Host-side, each core's sequence is rotated so its
own query rows are always rows 0:1024 — attention is permutation-invariant
over the key/value index, so this is exact — which lets q/gate/out read
slices of the full-sequence tensors with one uniform SPMD program.

Compute dtype is bf16 on the TensorEngine (the GAU branch contributes
~1e-10 of the output magnitude relative to the residual, so bf16 is far
inside the error budget); LayerNorm statistics and the final residual add
are fp32. Weights are cast to bf16 once and staged through DRAM so the
transposed layouts are produced by a few large XBAR DMAs; the cast traffic
is interleaved into compute phases to fill DMA slack.
"""

from contextlib import ExitStack

import numpy as np

import concourse.bacc as bacc
import concourse.mybir as mybir
import concourse.tile as tile
from concourse.bass_utils import run_bass_kernel_spmd
from concourse.masks import make_identity

dt = mybir.dt
AF = mybir.ActivationFunctionType
ALU = mybir.AluOpType
AX = mybir.AxisListType

B, S, D = 4, 2048, 768
H = 1536          # v / gate each get H columns of the 2*H hidden projection
QK = 128
N_CORES = 8
SO = S // 2       # own query rows per core
EPS = 1e-5

_CACHE: dict = {}
SIM_COMPAT = False  # lower Silu as Sigmoid+mul (CoreSim has no Silu LUT)


def _build(flags, reps=1):
    use_bqk, use_bg, use_bv, use_bout, use_lnw, use_lnb = flags
    nc = bacc.Bacc("TRN2", target_bir_lowering=False, num_devices=N_CORES)

    XK = nc.declare_dram_parameter("xk", [S, D], dt.float32, isOutput=False)
    WH = nc.declare_dram_parameter("wh", [2 * H, D], dt.float32, isOutput=False)
    WQKD = nc.declare_dram_parameter("wqk", [QK, D], dt.float32, isOutput=False)
    WOUT = nc.declare_dram_parameter("wout", [D, H], dt.float32, isOutput=False)
    G0 = nc.declare_dram_parameter("g0", [QK, 1], dt.float32, isOutput=False)
    B0 = nc.declare_dram_parameter("b0", [QK, 1], dt.float32, isOutput=False)
    G1 = nc.declare_dram_parameter("g1", [QK, 1], dt.float32, isOutput=False)
    B1 = nc.declare_dram_parameter("b1", [QK, 1], dt.float32, isOutput=False)
    BQK = nc.declare_dram_parameter("bqk", [QK, 1], dt.float32, isOutput=False)
    BG = nc.declare_dram_parameter("bg", [128, 12], dt.float32, isOutput=False)
    BV = nc.declare_dram_parameter("bv", [1, H], dt.float32, isOutput=False)
    BOUT = nc.declare_dram_parameter("bout", [1, D], dt.float32, isOutput=False)
    LNW = nc.declare_dram_parameter("lnw", [1, D], dt.float32, isOutput=False)
    LNB = nc.declare_dram_parameter("lnb", [1, D], dt.float32, isOutput=False)
    OUT = nc.declare_dram_parameter("out", [SO, D], dt.float32, isOutput=True)

    ND = D // 128    # 6 d-tiles
    NH = H // 128    # 12 h-tiles
    NJ = S // 128    # 16 j-tiles
    NI = SO // 128   # 8 own-row tiles
    bf16, f32 = dt.bfloat16, dt.float32

    with tile.TileContext(nc) as tc:
      for _rep in range(reps):
        top = ExitStack()
        consts = top.enter_context(tc.tile_pool(name=f"consts{_rep}", bufs=1))
        ident = consts.tile([128, 128], bf16)
        make_identity(nc, ident[:])

        sc = {}
        for nm, hdl in (("g0", G0), ("b0", B0), ("g1", G1), ("b1", B1),
                        ("bqk", BQK)):
            t = consts.tile([128, 1], f32, tag=f"sc_{nm}", name=f"sc_{nm}")
            nc.sync.dma_start(t[:], hdl[:])
            sc[nm] = t
        bg_sb = consts.tile([128, 12], f32)
        nc.sync.dma_start(bg_sb[:], BG[:])

        ones_row = None

        def bcast_row(hdl, n, nm, dtype=bf16):
            nonlocal ones_row
            if ones_row is None:
                ones_row = consts.tile([1, 128], bf16, tag="ones_row",
                                       name="ones_row")
                nc.vector.memset(ones_row[:], 1.0)
            row_f = consts.tile([1, n], f32, tag=f"rf_{nm}", name=f"rf_{nm}")
            nc.sync.dma_start(row_f[:], hdl[:])
            row_b = consts.tile([1, n], bf16, tag=f"rb_{nm}", name=f"rb_{nm}")
            nc.vector.tensor_copy(row_b[:], row_f[:])
            out_t = consts.tile([128, n], dtype, tag=f"bc_{nm}", name=f"bc_{nm}")
            with tc.tile_pool(name=f"bcps_{nm}{_rep}", bufs=1, space="PSUM") as pp:
                for c0 in range(0, n, 512):
                    cw = min(512, n - c0)
                    ps = pp.tile([128, 512], f32, tag="ps", name=f"bcp_{nm}{c0}")
                    nc.tensor.matmul(ps[:, :cw], ones_row[:],
                                     row_b[:, c0:c0 + cw], start=True, stop=True)
                    nc.vector.tensor_copy(out_t[:, c0:c0 + cw], ps[:, :cw])
            return out_t

        bv_bc = bcast_row(BV, H, "bv") if use_bv else None
        bout_bc = bcast_row(BOUT, D, "bout", f32) if use_bout else None
        lnw_bc = bcast_row(LNW, D, "lnw") if use_lnw else None
        lnb_bc = bcast_row(LNB, D, "lnb") if use_lnb else None

        # bf16 weight copies staged through DRAM; the transposed layouts are
        # then produced by a few large XBAR DMAs.
        dram = top.enter_context(tc.tile_pool(name=f"dram{_rep}", bufs=1,
                                              space="DRAM"))
        WHB = dram.tile([2 * H, D], bf16, tag="whb", name="WHB")
        WOB = dram.tile([D, H], bf16, tag="wob", name="WOB")
        WQB = dram.tile([QK, D], bf16, tag="wqb", name="WQB")

        # long-lived pools, opened in LIFO-compatible close order
        es_vg = ExitStack()
        vg_pool = es_vg.enter_context(tc.tile_pool(name=f"VgT{_rep}", bufs=1))
        VgT = [vg_pool.tile([128, SO], bf16, tag=f"vg{h}", name=f"VgT{h}")
               for h in range(NH)]
        es_wo = ExitStack()
        wo_pool = es_wo.enter_context(tc.tile_pool(name=f"woT{_rep}", bufs=1))
        W_oT = [wo_pool.tile([128, D], bf16, tag=f"w{h}", name=f"WoT{h}")
                for h in range(NH)]
        es_nkv = ExitStack()
        nkv_pool = es_nkv.enter_context(tc.tile_pool(name=f"nkvT{_rep}", bufs=1))
        normT = [nkv_pool.tile([128, S], bf16, tag=f"n{d}", name=f"nkvT{d}")
                 for d in range(ND)]
        es_kq = ExitStack()
        kqp = es_kq.enter_context(tc.tile_pool(name=f"kq{_rep}", bufs=1))
        kT = kqp.tile([128, S], bf16, tag="kT")
        qT = kqp.tile([128, SO], bf16, tag="qT")
        es_at = ExitStack()
        at_pool = es_at.enter_context(tc.tile_pool(name=f"AT{_rep}", bufs=1))
        AT = [at_pool.tile([128, SO], bf16, tag=f"a{j}", name=f"AT{j}")
              for j in range(NJ)]
        es_v = ExitStack()
        v_pool = es_v.enter_context(tc.tile_pool(name=f"vnat{_rep}", bufs=1))
        v = [v_pool.tile([128, H], bf16, tag=f"v{j}", name=f"vnat{j}")
             for j in range(NJ)]

        # weight-cast staging (closed after the joint A^T/v loop)
        es_wc = ExitStack()
        wc = es_wc.enter_context(tc.tile_pool(name=f"wcast{_rep}", bufs=3))

        def cast_tile(srch, dsth, rt, c0, nm):
            wf = wc.tile([128, D], f32, tag="wf", name=f"wf{nm}{rt}_{c0}")
            nc.sync.dma_start(wf[:], srch[rt * 128:(rt + 1) * 128, c0:c0 + D])
            wb = wc.tile([128, D], bf16, tag="wb", name=f"wb{nm}{rt}_{c0}")
            nc.vector.tensor_copy(wb[:], wf[:])
            nc.sync.dma_start(dsth[rt * 128:(rt + 1) * 128, c0:c0 + D], wb[:])

        es_wqk = ExitStack()
        p_wqk = es_wqk.enter_context(tc.tile_pool(name=f"wqkT{_rep}", bufs=1))
        wqkT = [p_wqk.tile([128, 128], bf16, tag=f"q{d}", name=f"wqkT{d}")
                for d in range(ND)]
        cast_tile(WQKD, WQB, 0, 0, "q")
        for d in range(ND):
            nc.sync.dma_start(wqkT[d][:], WQB[:, d * 128:(d + 1) * 128],
                              transpose=True)
        # v-half casts drain during LayerNorm; gate-half and W_out casts
        # during the joint A^T / v-projection loop.
        cast_ln = [("h", rt, 0) for rt in range(12)]
        cast_at = ([("h", rt, 0) for rt in range(12, 24)] +
                   [("o", rt, c0) for rt in range(6) for c0 in (0, D)])

        def drain_cast(lst, k):
            for _ in range(k):
                if not lst:
                    return
                nm, rt, c0 = lst.pop(0)
                cast_tile(WH if nm == "h" else WOUT,
                          WHB if nm == "h" else WOB, rt, c0, nm)

        def silu(out_ap, in_ap, pool, nm, bias=None):
            if not SIM_COMPAT:
                if bias is None:
                    nc.scalar.activation(out_ap, in_ap, AF.Silu)
                else:
                    nc.scalar.activation(out_ap, in_ap, AF.Silu, bias=bias)
                return
            sig = pool.tile([128, 512], f32, tag="sig", name=f"sig_{nm}")
            if bias is None:
                nc.scalar.activation(sig[:], in_ap, AF.Sigmoid)
                nc.vector.scalar_tensor_tensor(out_ap, in_ap, 0.0, sig[:],
                                               op0=ALU.add, op1=ALU.mult)
            else:
                nc.scalar.activation(sig[:], in_ap, AF.Sigmoid, bias=bias)
                nc.vector.scalar_tensor_tensor(out_ap, in_ap, bias, sig[:],
                                               op0=ALU.add, op1=ALU.mult)

        # ---- Phase 1: LayerNorm + transpose + qk projection, per row group
        es_mm = ExitStack()
        mm_ps = es_mm.enter_context(tc.tile_pool(name=f"mm_ps{_rep}", bufs=4,
                                                 space="PSUM"))
        es_ln = ExitStack()
        xpool = es_ln.enter_context(tc.tile_pool(name=f"xin{_rep}", bufs=4))
        lnp = es_ln.enter_context(tc.tile_pool(name=f"lnwork{_rep}", bufs=2))
        nbp = es_ln.enter_context(tc.tile_pool(name=f"nbuf{_rep}", bufs=5))
        stat = es_ln.enter_context(tc.tile_pool(name=f"stat{_rep}", bufs=12))
        zb1 = es_ln.enter_context(tc.tile_pool(name=f"zbuf1{_rep}", bufs=3))
        tp_ps = es_ln.enter_context(
            tc.tile_pool(name=f"tp_ps{_rep}", bufs=4, space="PSUM"))
        for g in range(NJ // 4):
            nbs = []
            for k in range(4):
                nt = g * 4 + k
                xt = xpool.tile([128, D], f32, tag="x", name=f"x{nt}")
                nc.sync.dma_start(xt[:], XK[nt * 128:(nt + 1) * 128, :])
                if cast_ln:
                    drain_cast(cast_ln, 2)
                s = stat.tile([128, 1], f32, tag="s", name=f"s{nt}")
                nc.vector.reduce_sum(s[:], xt[:], axis=AX.X)
                sq = lnp.tile([128, D], f32, tag="sq", name=f"sq{nt}")
                ss = stat.tile([128, 1], f32, tag="ss", name=f"ss{nt}")
                nc.scalar.activation(sq[:], xt[:], AF.Square, accum_out=ss[:])
                mu = stat.tile([128, 1], f32, tag="mu", name=f"mu{nt}")
                nc.scalar.mul(mu[:], s[:], 1.0 / D)
                # var = E[x^2] + eps - mu^2
                vv = stat.tile([128, 1], f32, tag="vv", name=f"vv{nt}")
                nc.vector.tensor_scalar(vv[:], ss[:], 1.0 / D, EPS,
                                        ALU.mult, ALU.add)
                msq = stat.tile([128, 1], f32, tag="msq", name=f"msq{nt}")
                nc.vector.scalar_tensor_tensor(msq[:], mu[:], 1.0, mu[:],
                                               op0=ALU.mult, op1=ALU.mult)
                var = stat.tile([128, 1], f32, tag="var", name=f"var{nt}")
                nc.vector.tensor_sub(var[:], vv[:], msq[:])
                sr = stat.tile([128, 1], f32, tag="sr", name=f"sr{nt}")
                nc.scalar.sqrt(sr[:], var[:])
                rstd = stat.tile([128, 1], f32, tag="rstd", name=f"rstd{nt}")
                nc.vector.reciprocal(rstd[:], sr[:])
                nb = nbp.tile([128, D], bf16, tag="nb", name=f"nb{nt}")
                if use_lnw or use_lnb:
                    nrm = lnp.tile([128, D], f32, tag="nrm", name=f"nrm{nt}")
                    nc.vector.tensor_scalar(nrm[:], xt[:], mu[:], rstd[:],
                                            ALU.subtract, ALU.mult)
                    if use_lnw and use_lnb:
                        nc.vector.tensor_mul(nb[:], nrm[:], lnw_bc[:])
                        nc.vector.tensor_add(nb[:], nb[:], lnb_bc[:])
                    elif use_lnw:
                        nc.vector.tensor_mul(nb[:], nrm[:], lnw_bc[:])
                    else:
                        nc.vector.tensor_add(nb[:], nrm[:], lnb_bc[:])
                else:
                    nc.vector.tensor_scalar(nb[:], xt[:], mu[:], rstd[:],
                                            ALU.subtract, ALU.mult)
                nbs.append(nb)
            for d in range(ND):
                ps = tp_ps.tile([128, 512], bf16, tag="tp", name=f"tp{g}_{d}")
                for k in range(4):
                    nc.tensor.transpose(ps[:, k * 128:(k + 1) * 128],
                                        nbs[k][:, d * 128:(d + 1) * 128],
                                        ident[:])
                if d % 2 == 0:
                    nc.scalar.copy(normT[d][:, g * 512:(g + 1) * 512], ps[:])
                else:
                    nc.vector.tensor_copy(normT[d][:, g * 512:(g + 1) * 512],
                                          ps[:])
            # qk projection for this 512-row chunk
            c = g
            ps = mm_ps.tile([128, 512], f32, tag="ps", name=f"qkps{c}")
            for d in range(ND):
                nc.tensor.matmul(ps[:], wqkT[d][:],
                                 normT[d][:, c * 512:(c + 1) * 512],
                                 start=(d == 0), stop=(d == ND - 1))
            zs = zb1.tile([128, 512], bf16, tag="z", name=f"z{c}")
            silu(zs[:], ps[:], zb1, f"z{c}",
                 bias=sc["bqk"][:] if use_bqk else None)
            nc.vector.tensor_scalar(kT[:, c * 512:(c + 1) * 512], zs[:],
                                    sc["g1"][:], sc["b1"][:],
                                    ALU.mult, ALU.add)
            if c < SO // 512:
                nc.vector.tensor_scalar(qT[:, c * 512:(c + 1) * 512],
                                        zs[:], sc["g0"][:], sc["b0"][:],
                                        ALU.mult, ALU.add)
        drain_cast(cast_ln, len(cast_ln))
        es_ln.close()
        es_wqk.close()

        # W_vT now (v-half of WHB is complete) so the joint loop can start
        es_wv = ExitStack()
        p_wv = es_wv.enter_context(tc.tile_pool(name=f"wvT{_rep}", bufs=1))
        W_vT = [p_wv.tile([128, H], bf16, tag=f"v{d}", name=f"WvT{d}")
                for d in range(ND)]
        for d in range(ND):
            nc.sync.dma_start(W_vT[d][:], WHB[0:H, d * 128:(d + 1) * 128],
                              transpose=True)

        # ---- Phase 2: joint loop over j: A^T[j] and v[j]
        with tc.tile_pool(name=f"rbuf{_rep}", bufs=3) as rb, \
                tc.tile_pool(name=f"vraw{_rep}", bufs=2) as vrp:
            for j in range(NJ):
                drain_cast(cast_at, 2)
                for c in range(SO // 512):
                    ps = mm_ps.tile([128, 512], f32, tag="ps",
                                    name=f"aps{j}_{c}")
                    nc.tensor.matmul(ps[:], kT[:, j * 128:(j + 1) * 128],
                                     qT[:, c * 512:(c + 1) * 512],
                                     start=True, stop=True)
                    r = rb.tile([128, 512], bf16, tag="r", name=f"r{j}_{c}")
                    nc.scalar.activation(r[:], ps[:], AF.Relu, scale=1.0 / S)
                    nc.vector.tensor_mul(AT[j][:, c * 512:(c + 1) * 512],
                                         r[:], r[:])
                for c in range(H // 512):
                    ps = mm_ps.tile([128, 512], f32, tag="ps",
                                    name=f"vps{j}_{c}")
                    for d in range(ND):
                        nc.tensor.matmul(ps[:],
                                         normT[d][:, j * 128:(j + 1) * 128],
                                         W_vT[d][:, c * 512:(c + 1) * 512],
                                         start=(d == 0), stop=(d == ND - 1))
                    if use_bv:
                        raw = vrp.tile([128, 512], f32, tag="vr",
                                       name=f"vr{j}_{c}")
                        nc.vector.tensor_add(raw[:], ps[:],
                                             bv_bc[:, c * 512:(c + 1) * 512])
                        silu(v[j][:, c * 512:(c + 1) * 512], raw[:], vrp,
                             f"v{j}_{c}")
                    else:
                        silu(v[j][:, c * 512:(c + 1) * 512], ps[:], vrp,
                             f"v{j}_{c}")
        es_wv.close()
        drain_cast(cast_at, len(cast_at))
        for h in range(NH):
            nc.sync.dma_start(W_oT[h][:], WOB[:, h * 128:(h + 1) * 128],
                              transpose=True)
        es_wc.close()

        # ---- Phase 3: V^T[h,i] = sum_j v[j][:,h].T @ A^T[j][:,i]
        for h in range(NH):
            for c in range(SO // 512):
                ps = mm_ps.tile([128, 512], f32, tag="ps", name=f"Vps{h}_{c}")
                for j in range(NJ):
                    nc.tensor.matmul(ps[:], v[j][:, h * 128:(h + 1) * 128],
                                     AT[j][:, c * 512:(c + 1) * 512],
                                     start=(j == 0), stop=(j == NJ - 1))
                nc.vector.tensor_copy(VgT[h][:, c * 512:(c + 1) * 512], ps[:])

        # ---- Phase 4: gate^T chunkwise, multiply into VgT
        es_wg = ExitStack()
        p_wg = es_wg.enter_context(tc.tile_pool(name=f"wgT{_rep}", bufs=1))
        W_gT = [p_wg.tile([128, H], bf16, tag=f"g{d}", name=f"WgT{d}")
                for d in range(ND)]
        for d in range(ND):
            nc.sync.dma_start(W_gT[d][:], WHB[H:2 * H, d * 128:(d + 1) * 128],
                              transpose=True)
        with tc.tile_pool(name=f"zg{_rep}", bufs=3) as zgp:
            for h in range(NH):
                for c in range(SO // 512):
                    ps = mm_ps.tile([128, 512], f32, tag="ps",
                                    name=f"gps{h}_{c}")
                    for d in range(ND):
                        nc.tensor.matmul(ps[:],
                                         W_gT[d][:, h * 128:(h + 1) * 128],
                                         normT[d][:, c * 512:(c + 1) * 512],
                                         start=(d == 0), stop=(d == ND - 1))
                    zg = zgp.tile([128, 512], bf16, tag="zg",
                                  name=f"zg{h}_{c}")
                    silu(zg[:], ps[:], zgp, f"zg{h}_{c}",
                         bias=bg_sb[:, h:h + 1] if use_bg else None)
                    nc.vector.tensor_mul(VgT[h][:, c * 512:(c + 1) * 512],
                                         VgT[h][:, c * 512:(c + 1) * 512],
                                         zg[:])
        es_wg.close()
        es_v.close()
        es_at.close()
        es_kq.close()
        es_nkv.close()

        # ---- Phase 5: out = VgT.T-blocks @ W_oT + x (+ b_out)
        with tc.tile_pool(name=f"xq2{_rep}", bufs=3) as xp2, \
                tc.tile_pool(name=f"obuf{_rep}", bufs=3) as op:
            for it in range(NI):
                xqt = xp2.tile([128, D], f32, tag="xq", name=f"xq{it}")
                nc.sync.dma_start(xqt[:], XK[it * 128:(it + 1) * 128, :])
                ob = op.tile([128, D], f32, tag="ob", name=f"ob{it}")
                cw = D // 2  # 384
                for c in range(2):
                    ps = mm_ps.tile([128, 512], f32, tag="ps",
                                    name=f"ops{it}_{c}")
                    for h in range(NH):
                        nc.tensor.matmul(ps[:, :cw],
                                         VgT[h][:, it * 128:(it + 1) * 128],
                                         W_oT[h][:, c * cw:(c + 1) * cw],
                                         start=(h == 0), stop=(h == NH - 1))
                    nc.vector.tensor_add(ob[:, c * cw:(c + 1) * cw],
                                         ps[:, :cw], xqt[:, c * cw:(c + 1) * cw])
                    if use_bout:
                        nc.vector.tensor_add(ob[:, c * cw:(c + 1) * cw],
                                             ob[:, c * cw:(c + 1) * cw],
                                             bout_bc[:, c * cw:(c + 1) * cw])
                nc.sync.dma_start(OUT[it * 128:(it + 1) * 128, :], ob[:])
        es_mm.close()
        es_wo.close()
        es_vg.close()
        top.close()

    nc.finalize()
    return nc


def _prep_in_maps(x, ln_w, ln_b, W_hidden, b_hidden, W_qk, b_qk, gamma, beta,
                  W_out, b_out):
    f32 = np.float32
    c = np.ascontiguousarray
    shared = {
        "wh": c(W_hidden, dtype=f32),
        "wqk": c(W_qk, dtype=f32),
        "wout": c(W_out, dtype=f32),
        "g0": c(gamma[0].reshape(QK, 1), dtype=f32),
        "b0": c(beta[0].reshape(QK, 1), dtype=f32),
        "g1": c(gamma[1].reshape(QK, 1), dtype=f32),
        "b1": c(beta[1].reshape(QK, 1), dtype=f32),
        "bqk": c(b_qk.reshape(QK, 1), dtype=f32),
        "bg": c(b_hidden[H:].reshape(12, 128).T, dtype=f32),
        "bv": c(b_hidden[:H].reshape(1, H), dtype=f32),
        "bout": c(b_out.reshape(1, D), dtype=f32),
        "lnw": c(ln_w.reshape(1, D), dtype=f32),
        "lnb": c(ln_b.reshape(1, D), dtype=f32),
    }
    in_maps = []
    for core in range(N_CORES):
        b, hf = core // 2, core % 2
        m = dict(shared)
        if hf == 0:
            m["xk"] = c(x[b], dtype=f32)
        else:
            m["xk"] = c(np.concatenate([x[b, SO:], x[b, :SO]], axis=0),
                        dtype=f32)
        in_maps.append(m)
    return in_maps


def _flags(ln_w, ln_b, b_hidden, b_qk, b_out):
    return (
        bool(np.any(b_qk)),
        bool(np.any(b_hidden[H:])),
        bool(np.any(b_hidden[:H])),
        bool(np.any(b_out)),
        bool(np.any(ln_w != 1.0)),
        bool(np.any(ln_b)),
    )


def get_program(inputs):
    flags = _flags(inputs["ln_w"], inputs["ln_b"], inputs["b_hidden"],
                   inputs["b_qk"], inputs["b_out"])
    key = (flags, SIM_COMPAT)
    if key not in _CACHE:
        _CACHE[key] = _build(flags)
    return _CACHE[key]


def kernel(x, ln_w, ln_b, W_hidden, b_hidden, W_qk, b_qk, gamma, beta,
           W_out, b_out):
    inputs = dict(x=np.asarray(x), ln_w=np.asarray(ln_w),
                  ln_b=np.asarray(ln_b), W_hidden=np.asarray(W_hidden),
                  b_hidden=np.asarray(b_hidden), W_qk=np.asarray(W_qk),
                  b_qk=np.asarray(b_qk), gamma=np.asarray(gamma),
                  beta=np.asarray(beta), W_out=np.asarray(W_out),
                  b_out=np.asarray(b_out))
    nc = get_program(inputs)
    in_maps = _prep_in_maps(**inputs)
    res = run_bass_kernel_spmd(nc, in_maps, core_ids=list(range(N_CORES)),
                               trace=False)
    out = np.empty((B, S, D), np.float32)
    for core in range(N_CORES):
        b, hf = core // 2, core % 2
        out[b, hf * SO:(hf + 1) * SO] = res.results[core]["out"]
    return out


# revision 25
# speedup vs baseline: 16727.4021x; 1.0088x over previous
"""GAU (Gated Attention Unit) Trainium2 kernel, 8-core SPMD.

Sharding: 2 cores per batch (B=4). Each core handles 1024 query rows of one
batch; the K/V path (LayerNorm + qk/v projections over the full 2048-row
sequence of that batch) is recomputed on both cores of a pair, which avoids
any cross-core collective. Host-side, each core's sequence is rotated so its
own query rows are always rows 0:1024 — attention is permutation-invariant
over the key/value index, so this is exact — which lets q/gate/out read
slices of the full-sequence tensors with one uniform SPMD program.

Compute dtype is bf16 on the TensorEngine (the GAU branch contributes
~1e-10 of the output magnitude relative to the residual, so bf16 is far
inside the error budget); LayerNorm statistics and the final residual add
are fp32. Weights are cast to bf16 once and staged through DRAM so the
transposed layouts are produced by a few large XBAR DMAs; the cast traffic
is interleaved into compute phases to fill DMA slack.
"""

from contextlib import ExitStack

import numpy as np

import concourse.bacc as bacc
import concourse.mybir as mybir
import concourse.tile as tile
from concourse.bass_utils import run_bass_kernel_spmd
from concourse.masks import make_identity

dt = mybir.dt
AF = mybir.ActivationFunctionType
ALU = mybir.AluOpType
AX = mybir.AxisListType

B, S, D = 4, 2048, 768
H = 1536          # v / gate each get H columns of the 2*H hidden projection
QK = 128
N_CORES = 8
SO = S // 2       # own query rows per core
EPS = 1e-5

_CACHE: dict = {}
SIM_COMPAT = False  # lower Silu as Sigmoid+mul (CoreSim has no Silu LUT)


def _build(flags, reps=1):
    use_bqk, use_bg, use_bv, use_bout, use_lnw, use_lnb = flags
    nc = bacc.Bacc("TRN2", target_bir_lowering=False, num_devices=N_CORES)

    XK = nc.declare_dram_parameter("xk", [S, D], dt.float32, isOutput=False)
    WH = nc.declare_dram_parameter("wh", [2 * H, D], dt.float32, isOutput=False)
    WQKD = nc.declare_dram_parameter("wqk", [QK, D], dt.float32, isOutput=False)
    WOUT = nc.declare_dram_parameter("wout", [D, H], dt.float32, isOutput=False)
    SCAL = nc.declare_dram_parameter("scal", [QK, 17], dt.float32,
                                     isOutput=False)
    BV = nc.declare_dram_parameter("bv", [1, H], dt.float32, isOutput=False)
    BOUT = nc.declare_dram_parameter("bout", [1, D], dt.float32, isOutput=False)
    LNW = nc.declare_dram_parameter("lnw", [1, D], dt.float32, isOutput=False)
    LNB = nc.declare_dram_parameter("lnb", [1, D], dt.float32, isOutput=False)
    OUT = nc.declare_dram_parameter("out", [SO, D], dt.float32, isOutput=True)

    ND = D // 128    # 6 d-tiles
    NH = H // 128    # 12 h-tiles
    NJ = S // 128    # 16 j-tiles
    NI = SO // 128   # 8 own-row tiles
    bf16, f32 = dt.bfloat16, dt.float32

    with tile.TileContext(nc) as tc:
      for _rep in range(reps):
        top = ExitStack()
        consts = top.enter_context(tc.tile_pool(name=f"consts{_rep}", bufs=1))
        ident = consts.tile([128, 128], bf16)
        make_identity(nc, ident[:])

        scal_sb = consts.tile([128, 17], f32, tag="scal", name="scal")
        nc.sync.dma_start(scal_sb[:], SCAL[:])
        sc = {nm: scal_sb[:, i:i + 1]
              for i, nm in enumerate(("g0", "b0", "g1", "b1", "bqk"))}
        bg_sb = scal_sb[:, 5:17]

        ones_row = None

        def bcast_row(hdl, n, nm, dtype=bf16):
            nonlocal ones_row
            if ones_row is None:
                ones_row = consts.tile([1, 128], bf16, tag="ones_row",
                                       name="ones_row")
                nc.vector.memset(ones_row[:], 1.0)
            row_f = consts.tile([1, n], f32, tag=f"rf_{nm}", name=f"rf_{nm}")
            nc.sync.dma_start(row_f[:], hdl[:])
            row_b = consts.tile([1, n], bf16, tag=f"rb_{nm}", name=f"rb_{nm}")
            nc.vector.tensor_copy(row_b[:], row_f[:])
            out_t = consts.tile([128, n], dtype, tag=f"bc_{nm}", name=f"bc_{nm}")
            with tc.tile_pool(name=f"bcps_{nm}{_rep}", bufs=1, space="PSUM") as pp:
                for c0 in range(0, n, 512):
                    cw = min(512, n - c0)
                    ps = pp.tile([128, 512], f32, tag="ps", name=f"bcp_{nm}{c0}")
                    nc.tensor.matmul(ps[:, :cw], ones_row[:],
                                     row_b[:, c0:c0 + cw], start=True, stop=True)
                    nc.vector.tensor_copy(out_t[:, c0:c0 + cw], ps[:, :cw])
            return out_t

        bv_bc = bcast_row(BV, H, "bv") if use_bv else None
        bout_bc = bcast_row(BOUT, D, "bout", f32) if use_bout else None
        lnw_bc = bcast_row(LNW, D, "lnw") if use_lnw else None
        lnb_bc = bcast_row(LNB, D, "lnb") if use_lnb else None

        # bf16 weight copies staged through DRAM; the transposed layouts are
        # then produced by a few large XBAR DMAs.
        dram = top.enter_context(tc.tile_pool(name=f"dram{_rep}", bufs=1,
                                              space="DRAM"))
        WHB = dram.tile([2 * H, D], bf16, tag="whb", name="WHB")
        WOB = dram.tile([D, H], bf16, tag="wob", name="WOB")
        WQB = dram.tile([QK, D], bf16, tag="wqb", name="WQB")

        # long-lived pools, opened in LIFO-compatible close order
        es_vg = ExitStack()
        vg_pool = es_vg.enter_context(tc.tile_pool(name=f"VgT{_rep}", bufs=1))
        VgT = [vg_pool.tile([128, SO], bf16, tag=f"vg{h}", name=f"VgT{h}")
               for h in range(NH)]
        es_wo = ExitStack()
        wo_pool = es_wo.enter_context(tc.tile_pool(name=f"woT{_rep}", bufs=1))
        W_oT = [wo_pool.tile([128, D], bf16, tag=f"w{h}", name=f"WoT{h}")
                for h in range(NH)]
        es_nkv = ExitStack()
        nkv_pool = es_nkv.enter_context(tc.tile_pool(name=f"nkvT{_rep}", bufs=1))
        normT = [nkv_pool.tile([128, S], bf16, tag=f"n{d}", name=f"nkvT{d}")
                 for d in range(ND)]
        es_kq = ExitStack()
        kqp = es_kq.enter_context(tc.tile_pool(name=f"kq{_rep}", bufs=1))
        kT = kqp.tile([128, S], bf16, tag="kT")
        qT = kqp.tile([128, SO], bf16, tag="qT")
        es_at = ExitStack()
        at_pool = es_at.enter_context(tc.tile_pool(name=f"AT{_rep}", bufs=1))
        AT = [at_pool.tile([128, SO], bf16, tag=f"a{j}", name=f"AT{j}")
              for j in range(NJ)]
        es_v = ExitStack()
        v_pool = es_v.enter_context(tc.tile_pool(name=f"vnat{_rep}", bufs=1))
        v = [v_pool.tile([128, H], bf16, tag=f"v{j}", name=f"vnat{j}")
             for j in range(NJ)]

        # weight-cast staging (closed after the joint A^T/v loop)
        es_wc = ExitStack()
        wc = es_wc.enter_context(tc.tile_pool(name=f"wcast{_rep}", bufs=3))

        def cast_tile(srch, dsth, rt, c0, nm):
            wf = wc.tile([128, D], f32, tag="wf", name=f"wf{nm}{rt}_{c0}")
            nc.sync.dma_start(wf[:], srch[rt * 128:(rt + 1) * 128, c0:c0 + D])
            wb = wc.tile([128, D], bf16, tag="wb", name=f"wb{nm}{rt}_{c0}")
            nc.vector.tensor_copy(wb[:], wf[:])
            nc.sync.dma_start(dsth[rt * 128:(rt + 1) * 128, c0:c0 + D], wb[:])

        es_wqk = ExitStack()
        p_wqk = es_wqk.enter_context(tc.tile_pool(name=f"wqkT{_rep}", bufs=1))
        wqkT = [p_wqk.tile([128, 128], bf16, tag=f"q{d}", name=f"wqkT{d}")
                for d in range(ND)]
        cast_tile(WQKD, WQB, 0, 0, "q")
        for d in range(ND):
            nc.sync.dma_start(wqkT[d][:], WQB[:, d * 128:(d + 1) * 128],
                              transpose=True)
        # v-half casts drain during LayerNorm; gate-half and W_out casts
        # during the joint A^T / v-projection loop.
        cast_ln = [("h", rt, 0) for rt in range(12)]
        cast_at = ([("h", rt, 0) for rt in range(12, 24)] +
                   [("o", rt, c0) for rt in range(6) for c0 in (0, D)])

        def drain_cast(lst, k):
            for _ in range(k):
                if not lst:
                    return
                nm, rt, c0 = lst.pop(0)
                cast_tile(WH if nm == "h" else WOUT,
                          WHB if nm == "h" else WOB, rt, c0, nm)

        def silu(out_ap, in_ap, pool, nm, bias=None):
            if not SIM_COMPAT:
                if bias is None:
                    nc.scalar.activation(out_ap, in_ap, AF.Silu)
                else:
                    nc.scalar.activation(out_ap, in_ap, AF.Silu, bias=bias)
                return
            sig = pool.tile([128, 512], f32, tag="sig", name=f"sig_{nm}")
            if bias is None:
                nc.scalar.activation(sig[:], in_ap, AF.Sigmoid)
                nc.vector.scalar_tensor_tensor(out_ap, in_ap, 0.0, sig[:],
                                               op0=ALU.add, op1=ALU.mult)
            else:
                nc.scalar.activation(sig[:], in_ap, AF.Sigmoid, bias=bias)
                nc.vector.scalar_tensor_tensor(out_ap, in_ap, bias, sig[:],
                                               op0=ALU.add, op1=ALU.mult)

        # ---- Phase 1: LayerNorm + transpose + qk projection, per row group
        es_mm = ExitStack()
        mm_ps = es_mm.enter_context(tc.tile_pool(name=f"mm_ps{_rep}", bufs=4,
                                                 space="PSUM"))
        es_ln = ExitStack()
        xpool = es_ln.enter_context(tc.tile_pool(name=f"xin{_rep}", bufs=4))
        lnp = es_ln.enter_context(tc.tile_pool(name=f"lnwork{_rep}", bufs=2))
        nbp = es_ln.enter_context(tc.tile_pool(name=f"nbuf{_rep}", bufs=5))
        stat = es_ln.enter_context(tc.tile_pool(name=f"stat{_rep}", bufs=12))
        zb1 = es_ln.enter_context(tc.tile_pool(name=f"zbuf1{_rep}", bufs=3))
        tp_ps = es_ln.enter_context(
            tc.tile_pool(name=f"tp_ps{_rep}", bufs=4, space="PSUM"))
        for g in range(NJ // 4):
            nbs = []
            for k in range(4):
                nt = g * 4 + k
                xt = xpool.tile([128, D], f32, tag="x", name=f"x{nt}")
                nc.sync.dma_start(xt[:], XK[nt * 128:(nt + 1) * 128, :])
                if cast_ln:
                    drain_cast(cast_ln, 2)
                s = stat.tile([128, 1], f32, tag="s", name=f"s{nt}")
                nc.vector.reduce_sum(s[:], xt[:], axis=AX.X)
                sq = lnp.tile([128, D], f32, tag="sq", name=f"sq{nt}")
                ss = stat.tile([128, 1], f32, tag="ss", name=f"ss{nt}")
                nc.scalar.activation(sq[:], xt[:], AF.Square, accum_out=ss[:])
                mu = stat.tile([128, 1], f32, tag="mu", name=f"mu{nt}")
                nc.scalar.mul(mu[:], s[:], 1.0 / D)
                # var = E[x^2] + eps - mu^2
                vv = stat.tile([128, 1], f32, tag="vv", name=f"vv{nt}")
                nc.vector.tensor_scalar(vv[:], ss[:], 1.0 / D, EPS,
                                        ALU.mult, ALU.add)
                msq = stat.tile([128, 1], f32, tag="msq", name=f"msq{nt}")
                nc.vector.scalar_tensor_tensor(msq[:], mu[:], 1.0, mu[:],
                                               op0=ALU.mult, op1=ALU.mult)
                var = stat.tile([128, 1], f32, tag="var", name=f"var{nt}")
                nc.vector.tensor_sub(var[:], vv[:], msq[:])
                sr = stat.tile([128, 1], f32, tag="sr", name=f"sr{nt}")
                nc.scalar.sqrt(sr[:], var[:])
                rstd = stat.tile([128, 1], f32, tag="rstd", name=f"rstd{nt}")
                nc.vector.reciprocal(rstd[:], sr[:])
                nb = nbp.tile([128, D], bf16, tag="nb", name=f"nb{nt}")
                if use_lnw or use_lnb:
                    nrm = lnp.tile([128, D], f32, tag="nrm", name=f"nrm{nt}")
                    nc.vector.tensor_scalar(nrm[:], xt[:], mu[:], rstd[:],
                                            ALU.subtract, ALU.mult)
                    if use_lnw and use_lnb:
                        nc.vector.tensor_mul(nb[:], nrm[:], lnw_bc[:])
                        nc.vector.tensor_add(nb[:], nb[:], lnb_bc[:])
                    elif use_lnw:
                        nc.vector.tensor_mul(nb[:], nrm[:], lnw_bc[:])
                    else:
                        nc.vector.tensor_add(nb[:], nrm[:], lnb_bc[:])
                else:
                    nc.vector.tensor_scalar(nb[:], xt[:], mu[:], rstd[:],
                                            ALU.subtract, ALU.mult)
                nbs.append(nb)
            for d in range(ND):
                ps = tp_ps.tile([128, 512], bf16, tag="tp", name=f"tp{g}_{d}")
                for k in range(4):
                    nc.tensor.transpose(ps[:, k * 128:(k + 1) * 128],
                                        nbs[k][:, d * 128:(d + 1) * 128],
                                        ident[:])
                if d % 2 == 0:
                    nc.scalar.copy(normT[d][:, g * 512:(g + 1) * 512], ps[:])
                else:
                    nc.vector.tensor_copy(normT[d][:, g * 512:(g + 1) * 512],
                                          ps[:])
            # qk projection for this 512-row chunk
            c = g
            ps = mm_ps.tile([128, 512], f32, tag="ps", name=f"qkps{c}")
            for d in range(ND):
                nc.tensor.matmul(ps[:], wqkT[d][:],
                                 normT[d][:, c * 512:(c + 1) * 512],
                                 start=(d == 0), stop=(d == ND - 1))
            zs = zb1.tile([128, 512], bf16, tag="z", name=f"z{c}")
            silu(zs[:], ps[:], zb1, f"z{c}",
                 bias=sc["bqk"][:] if use_bqk else None)
            nc.vector.tensor_scalar(kT[:, c * 512:(c + 1) * 512], zs[:],
                                    sc["g1"][:], sc["b1"][:],
                                    ALU.mult, ALU.add)
            if c < SO // 512:
                nc.vector.tensor_scalar(qT[:, c * 512:(c + 1) * 512],
                                        zs[:], sc["g0"][:], sc["b0"][:],
                                        ALU.mult, ALU.add)
        drain_cast(cast_ln, len(cast_ln))
        es_ln.close()
        es_wqk.close()

        # W_vT now (v-half of WHB is complete) so the joint loop can start
        es_wv = ExitStack()
        p_wv = es_wv.enter_context(tc.tile_pool(name=f"wvT{_rep}", bufs=1))
        W_vT = [p_wv.tile([128, H], bf16, tag=f"v{d}", name=f"WvT{d}")
                for d in range(ND)]
        for d in range(ND):
            nc.sync.dma_start(W_vT[d][:], WHB[0:H, d * 128:(d + 1) * 128],
                              transpose=True)

        # ---- Phase 2: joint loop over j: A^T[j] and v[j]
        with tc.tile_pool(name=f"rbuf{_rep}", bufs=3) as rb, \
                tc.tile_pool(name=f"vraw{_rep}", bufs=2) as vrp:
            for j in range(NJ):
                drain_cast(cast_at, 2)
                for c in range(SO // 512):
                    ps = mm_ps.tile([128, 512], f32, tag="ps",
                                    name=f"aps{j}_{c}")
                    nc.tensor.matmul(ps[:], kT[:, j * 128:(j + 1) * 128],
                                     qT[:, c * 512:(c + 1) * 512],
                                     start=True, stop=True)
                    r = rb.tile([128, 512], bf16, tag="r", name=f"r{j}_{c}")
                    nc.scalar.activation(r[:], ps[:], AF.Relu, scale=1.0 / S)
                    nc.vector.tensor_mul(AT[j][:, c * 512:(c + 1) * 512],
                                         r[:], r[:])
                for c in range(H // 512):
                    ps = mm_ps.tile([128, 512], f32, tag="ps",
                                    name=f"vps{j}_{c}")
                    for d in range(ND):
                        nc.tensor.matmul(ps[:],
                                         normT[d][:, j * 128:(j + 1) * 128],
                                         W_vT[d][:, c * 512:(c + 1) * 512],
                                         start=(d == 0), stop=(d == ND - 1))
                    if use_bv:
                        raw = vrp.tile([128, 512], f32, tag="vr",
                                       name=f"vr{j}_{c}")
                        nc.vector.tensor_add(raw[:], ps[:],
                                             bv_bc[:, c * 512:(c + 1) * 512])
                        silu(v[j][:, c * 512:(c + 1) * 512], raw[:], vrp,
                             f"v{j}_{c}")
                    else:
                        silu(v[j][:, c * 512:(c + 1) * 512], ps[:], vrp,
                             f"v{j}_{c}")
        es_wv.close()
        drain_cast(cast_at, len(cast_at))
        for h in range(NH):
            nc.sync.dma_start(W_oT[h][:], WOB[:, h * 128:(h + 1) * 128],
                              transpose=True)
        es_wc.close()

        # ---- Phase 3: V^T[h,i] = sum_j v[j][:,h].T @ A^T[j][:,i]
        for h in range(NH):
            for c in range(SO // 512):
                ps = mm_ps.tile([128, 512], f32, tag="ps", name=f"Vps{h}_{c}")
                for j in range(NJ):
                    nc.tensor.matmul(ps[:], v[j][:, h * 128:(h + 1) * 128],
                                     AT[j][:, c * 512:(c + 1) * 512],
                                     start=(j == 0), stop=(j == NJ - 1))
                nc.vector.tensor_copy(VgT[h][:, c * 512:(c + 1) * 512], ps[:])

        # ---- Phase 4: gate^T chunkwise, multiply into VgT
        es_wg = ExitStack()
        p_wg = es_wg.enter_context(tc.tile_pool(name=f"wgT{_rep}", bufs=1))
        W_gT = [p_wg.tile([128, H], bf16, tag=f"g{d}", name=f"WgT{d}")
                for d in range(ND)]
        for d in range(ND):
            nc.sync.dma_start(W_gT[d][:], WHB[H:2 * H, d * 128:(d + 1) * 128],
                              transpose=True)
        with tc.tile_pool(name=f"zg{_rep}", bufs=3) as zgp:
            for h in range(NH):
                for c in range(SO // 512):
                    ps = mm_ps.tile([128, 512], f32, tag="ps",
                                    name=f"gps{h}_{c}")
                    for d in range(ND):
                        nc.tensor.matmul(ps[:],
                                         W_gT[d][:, h * 128:(h + 1) * 128],
                                         normT[d][:, c * 512:(c + 1) * 512],
                                         start=(d == 0), stop=(d == ND - 1))
                    zg = zgp.tile([128, 512], bf16, tag="zg",
                                  name=f"zg{h}_{c}")
                    silu(zg[:], ps[:], zgp, f"zg{h}_{c}",
                         bias=bg_sb[:, h:h + 1] if use_bg else None)
                    nc.vector.tensor_mul(VgT[h][:, c * 512:(c + 1) * 512],
                                         VgT[h][:, c * 512:(c + 1) * 512],
                                         zg[:])
        es_wg.close()
        es_v.close()
        es_at.close()
        es_kq.close()
        es_nkv.close()

        # ---- Phase 5: out = VgT.T-blocks @ W_oT + x (+ b_out)
        with tc.tile_pool(name=f"xq2{_rep}", bufs=3) as xp2, \
                tc.tile_pool(name=f"obuf{_rep}", bufs=3) as op:
            for it in range(NI):
                xqt = xp2.tile([128, D], f32, tag="xq", name=f"xq{it}")
                nc.sync.dma_start(xqt[:], XK[it * 128:(it + 1) * 128, :])
                ob = op.tile([128, D], f32, tag="ob", name=f"ob{it}")
                cw = D // 2  # 384
                for c in range(2):
                    ps = mm_ps.tile([128, 512], f32, tag="ps",
                                    name=f"ops{it}_{c}")
                    for h in range(NH):
                        nc.tensor.matmul(ps[:, :cw],
                                         VgT[h][:, it * 128:(it + 1) * 128],
                                         W_oT[h][:, c * cw:(c + 1) * cw],
                                         start=(h == 0), stop=(h == NH - 1))
                    nc.vector.tensor_add(ob[:, c * cw:(c + 1) * cw],
                                         ps[:, :cw], xqt[:, c * cw:(c + 1) * cw])
                    if use_bout:
                        nc.vector.tensor_add(ob[:, c * cw:(c + 1) * cw],
                                             ob[:, c * cw:(c + 1) * cw],
                                             bout_bc[:, c * cw:(c + 1) * cw])
                nc.sync.dma_start(OUT[it * 128:(it + 1) * 128, :], ob[:])
        es_mm.close()
        es_wo.close()
        es_vg.close()
        top.close()

    nc.finalize()
    return nc


def _prep_in_maps(x, ln_w, ln_b, W_hidden, b_hidden, W_qk, b_qk, gamma, beta,
                  W_out, b_out):
    f32 = np.float32
    c = np.ascontiguousarray
    shared = {
        "wh": c(W_hidden, dtype=f32),
        "wqk": c(W_qk, dtype=f32),
        "wout": c(W_out, dtype=f32),
        "scal": c(np.concatenate(
            [gamma[0].reshape(QK, 1), beta[0].reshape(QK, 1),
             gamma[1].reshape(QK, 1), beta[1].reshape(QK, 1),
             b_qk.reshape(QK, 1), b_hidden[H:].reshape(12, 128).T],
            axis=1), dtype=f32),
        "bv": c(b_hidden[:H].reshape(1, H), dtype=f32),
        "bout": c(b_out.reshape(1, D), dtype=f32),
        "lnw": c(ln_w.reshape(1, D), dtype=f32),
        "lnb": c(ln_b.reshape(1, D), dtype=f32),
    }
    in_maps = []
    for core in range(N_CORES):
        b, hf = core // 2, core % 2
        m = dict(shared)
        if hf == 0:
            m["xk"] = c(x[b], dtype=f32)
        else:
            m["xk"] = c(np.concatenate([x[b, SO:], x[b, :SO]], axis=0),
                        dtype=f32)
        in_maps.append(m)
    return in_maps


def _flags(ln_w, ln_b, b_hidden, b_qk, b_out):
    return (
        bool(np.any(b_qk)),
        bool(np.any(b_hidden[H:])),
        bool(np.any(b_hidden[:H])),
        bool(np.any(b_out)),
        bool(np.any(ln_w != 1.0)),
        bool(np.any(ln_b)),
    )


def get_program(inputs):
    flags = _flags(inputs["ln_w"], inputs["ln_b"], inputs["b_hidden"],
                   inputs["b_qk"], inputs["b_out"])
    key = (flags, SIM_COMPAT)
    if key not in _CACHE:
        _CACHE[key] = _build(flags)
    return _CACHE[key]


def kernel(x, ln_w, ln_b, W_hidden, b_hidden, W_qk, b_qk, gamma, beta,
           W_out, b_out):
    inputs = dict(x=np.asarray(x), ln_w=np.asarray(ln_w),
                  ln_b=np.asarray(ln_b), W_hidden=np.asarray(W_hidden),
                  b_hidden=np.asarray(b_hidden), W_qk=np.asarray(W_qk),
                  b_qk=np.asarray(b_qk), gamma=np.asarray(gamma),
                  beta=np.asarray(beta), W_out=np.asarray(W_out),
                  b_out=np.asarray(b_out))
    nc = get_program(inputs)
    in_maps = _prep_in_maps(**inputs)
    res = run_bass_kernel_spmd(nc, in_maps, core_ids=list(range(N_CORES)),
                               trace=False)
    out = np.empty((B, S, D), np.float32)
    for core in range(N_CORES):
        b, hf = core // 2, core % 2
        out[b, hf * SO:(hf + 1) * SO] = res.results[core]["out"]
    return out
